# revision 19
# baseline (speedup 1.0000x reference)
"""ChannelBlock (dense transformer block with channel/cross-covariance attention)
Trainium2 Bass kernel, data-parallel over batch across 8 NeuronCores.

Contract: kernel(**inputs) takes FULL unsharded inputs (np arrays), returns the
FULL output [8, 4096, 256] float32.

v2 design notes (per-core, one batch element):
 - activation transposes ride the DMA xbar (dma_start_transpose, bf16),
   freeing the PE of ~30us of identity-matmul transposes.
 - phase A (LN1 + kv + q) runs bf16 with k/v/q scales folded into the
   weights host-side, so the psum evictions are pure casts.
 - channel-attention accumulation (k^T v), proj, fc1 and fc2 run fp8e4
   with DoubleRow (2 fp8 weights per PE cell).
 - fc2 is computed feature-major (stationary = w2 slices, reused across all
   tokens: 8 ldweights total) and the result is DMA-transposed back.
 - LN rstd = exp(-0.5*ln(var+eps)): ln+exp live in one ACT table set, so the
   only table switch in the whole kernel is to gelu for the MLP.
"""

import os

import numpy as np

import concourse.bass as bass
import concourse.bass_utils as _bu
import concourse.tile as tile
from concourse import mybir
from concourse.bass_utils import run_bass_kernel_spmd
from concourse.vector_clock import ScopedClock
import bass_rust

# ----------------------------------------------------------------------------
# Workaround: this container's walrus (CoreV3) only supports ONE sync-wait
# command on TPB_CTRL instructions (Drain).  Tile's kernel-tail drain piles all
# outstanding proc waits onto a single Drain -> split into a chain of Drains
# with one wait each.
# ----------------------------------------------------------------------------
_MAX_DRAIN_WAITS = 1


def _patched_drain_and_barrier(self, tick_clock, wait_clock):
    drain_inst = self.nc.sync.drain()
    wait_clock.add_sem_waits(
        drain_inst.ins, ScopedClock({None: tick_clock.global_clock})
    )
    mi = drain_inst.ins
    si = mi.sync_info
    waits = list(si.on_wait) if si else []
    if len(waits) > _MAX_DRAIN_WAITS:
        mi.sync_info = bass_rust.SyncInfo(
            on_wait=waits[:_MAX_DRAIN_WAITS], on_update=list(si.on_update)
        )
        for i in range(_MAX_DRAIN_WAITS, len(waits), _MAX_DRAIN_WAITS):
            extra = self.nc.sync.drain()
            extra.ins.sync_info = bass_rust.SyncInfo(
                on_wait=waits[i : i + _MAX_DRAIN_WAITS], on_update=[]
            )
    self.nc.all_engine_barrier()
    popped = self.nc._tile_sem_poison_stack.pop()
    assert popped is self._sem_poison
    self.nc.clear_and_free_semaphores(list(self.sems.allocated().values()))
    self.nc.all_engine_barrier()


tile.TileContext._drain_and_barrier = _patched_drain_and_barrier

_nop_counter = [0]


def _split_sync_waits(nc, cap=1):
    """Walrus in this container rejects instructions with more than `cap`
    sync-wait commands.  Hoist excess waits onto same-engine NOPs inserted
    immediately before the instruction (engine streams are in-order, so the
    semantics are unchanged)."""
    for f in nc.m.functions:
        for blk in f.blocks:
            changed = False
            new = []
            for inst in blk.instructions:
                si = inst.sync_info
                waits = list(si.on_wait) if si is not None else []
                # ldw-opt rejects Ldweights carrying sync waits; hoist them.
                is_ldw = inst.__class__.__name__ == "InstLdweights"
                eff_cap = 0 if (is_ldw and waits) else cap
                if len(waits) > eff_cap:
                    if is_ldw:
                        excess, keep = waits, []
                    else:
                        excess, keep = waits[:-cap], waits[-cap:]
                    for j in range(0, len(excess), cap):
                        _nop_counter[0] += 1
                        nop = mybir.InstNoOp(
                            name=f"NW-{_nop_counter[0]}", ins=[], outs=[]
                        )
                        nop.engine = inst.engine
                        nop.sync_info = bass_rust.SyncInfo(
                            on_wait=excess[j : j + cap], on_update=[]
                        )
                        new.append(nop)
                    inst.sync_info = bass_rust.SyncInfo(
                        on_wait=keep, on_update=list(si.on_update)
                    )
                    changed = True
                new.append(inst)
            if changed:
                blk.instructions = new


# ----------------------------------------------------------------------------
# Problem constants (hardcoded per the task contract)
# ----------------------------------------------------------------------------
B = 8
N = 4096
C = 256
H = 8
HD = C // H  # 32
HID = 1024
EPS = 1e-5
P = 128
NTILES = N // P  # 32
NG = NTILES // 4  # 8 groups of 4 tiles (512 tokens each)

F32 = mybir.dt.float32
BF16 = mybir.dt.bfloat16
FP8 = mybir.dt.float8e4
NP_BF16 = mybir.dt.np(BF16)
NP_FP8 = mybir.dt.np(FP8)

AF = mybir.ActivationFunctionType
ALU = mybir.AluOpType
AX = mybir.AxisListType
DR = mybir.MatmulPerfMode.DoubleRow

# activation scales (power-of-two; fp8 range management only)
S_K = 64.0
S_V = 16.0
S_Q = 16.0
S_X2 = 16.0
S_E = 256.0
INV_ATTN = 1.0 / (S_K * S_V)
INV_PROJ = 1.0 / (S_Q * S_E)
LN_SX2 = float(np.log(S_X2))


def _build_nc(has_bkv, has_bproj, s_w1, s_w2):
    nc = bass.Bass()

    # ---- DRAM I/O ----
    x_d = nc.declare_dram_parameter("x", [N, C], F32, isOutput=False)
    wkv_d = nc.declare_dram_parameter("wkv", [2, P, 2 * C], BF16, isOutput=False)
    wq_d = nc.declare_dram_parameter("wq", [2, P, C], BF16, isOutput=False)
    wproj_d = nc.declare_dram_parameter("wproj", [2, P, C], BF16, isOutput=False)
    w1_d = nc.declare_dram_parameter("w1", [2, P, HID], FP8, isOutput=False)
    w2_d = nc.declare_dram_parameter("w2", [8, P, C], FP8, isOutput=False)
    bq_d = nc.declare_dram_parameter("bq", [2, P, 1], F32, isOutput=False)
    b1_d = nc.declare_dram_parameter("b1", [8, P, 1], F32, isOutput=False)
    bfc2_d = nc.declare_dram_parameter("bfc2", [2, P, 1], F32, isOutput=False)
    bkv_d = nc.declare_dram_parameter("bkv", [1, 2 * C], BF16, isOutput=False)
    bproj_d = nc.declare_dram_parameter("bproj", [1, C], BF16, isOutput=False)
    out_d = nc.declare_dram_parameter("out", [N, C], F32, isOutput=True)
    DBG = os.environ.get("BASS_DBG", "0") == "1"
    if DBG:
        dbg_d = {
            k: nc.declare_dram_parameter(f"dbg_{k}", shp, dt, isOutput=True)
            for k, (shp, dt) in {
                "xhT": ([P, 2 * N], BF16),
                "qT": ([P, 2 * N], FP8),
                "E": ([P, 2 * C], FP8),
                "h1": ([P, NTILES * C], F32),
                "x2T": ([P, 2 * N], FP8),
                "g1T": ([P, 8 * N], FP8),
                "mT": ([P, 2 * N], BF16),
                "w18": ([P, 2 * HID], FP8),
                "w28": ([P, 8 * C], FP8),
            }.items()
        }

    with tile.TileContext(nc) as tc:
        import contextlib

        ctx = contextlib.ExitStack()
        with ctx:
            const = ctx.enter_context(tc.tile_pool(name="const", bufs=1))
            xres = ctx.enter_context(tc.tile_pool(name="xres", bufs=1))
            stats = ctx.enter_context(tc.tile_pool(name="stats", bufs=4))
            work = ctx.enter_context(tc.tile_pool(name="work", bufs=4))
            kvp = ctx.enter_context(tc.tile_pool(name="kvp", bufs=3))
            outp = ctx.enter_context(tc.tile_pool(name="outp", bufs=3))

            # ---- residents ----
            x_sb = xres.tile([P, NTILES, C], F32)  # raw x, token-major
            h1_sb = xres.tile([P, NTILES, C], F32)  # x + attn, token-major
            xhT = xres.tile([P, 2, N], BF16)  # LN1(x)^T  (feature-major)
            qT8 = xres.tile([P, 2, N], FP8)  # (q*S_Q)^T
            x2T = xres.tile([P, 2, N], BF16)  # (LN2(h1)*S_X2)^T
            x2T8 = xres.tile([P, 2, N], FP8)
            g1T8 = xres.tile([P, 8, N], FP8)  # gelu(fc1)^T
            mTb = xres.tile([P, 2, N], BF16)  # fc2 out, feature-major
            mv32 = xres.tile([P, NTILES, 2], F32)
            rs32 = xres.tile([P, NTILES], F32)

            # ---- input DMAs: x first (compute starts on it), weights after --
            for g in range(NG):
                nc.sync.dma_start(
                    out=x_sb[:, 4 * g : 4 * g + 4, :],
                    in_=x_d[512 * g : 512 * (g + 1), :].rearrange(
                        "(s p) c -> p s c", p=P
                    ),
                )
            wkv = const.tile([P, 2, 2 * C], BF16)
            wq = const.tile([P, 2, C], BF16)
            wproj = const.tile([P, 2, C], BF16)
            bq = const.tile([P, 2], F32)
            nc.sync.dma_start(out=wkv[:], in_=wkv_d.rearrange("c p f -> p c f"))
            nc.sync.dma_start(out=wq[:], in_=wq_d.rearrange("c p f -> p c f"))
            nc.sync.dma_start(out=wproj[:], in_=wproj_d.rearrange("c p f -> p c f"))
            for c in range(2):
                nc.sync.dma_start(out=bq[:, c : c + 1], in_=bq_d[c])
            w18 = const.tile([P, 2, HID], FP8)
            w28 = const.tile([P, 8, C], FP8)
            b1 = const.tile([P, 8], F32)
            bfc2 = const.tile([P, 2], F32)
            nc.sync.dma_start(out=w18[:], in_=w1_d.rearrange("c p f -> p c f"))
            nc.sync.dma_start(out=w28[:], in_=w2_d.rearrange("c p f -> p c f"))
            for c in range(8):
                nc.sync.dma_start(out=b1[:, c : c + 1], in_=b1_d[c])
            for c in range(2):
                nc.sync.dma_start(out=bfc2[:, c : c + 1], in_=bfc2_d[c])
            ones_row = const.tile([1, P], BF16)
            nc.vector.memset(ones_row[:], 1.0)
            eps_t = const.tile([P, 1], F32)
            nc.vector.memset(eps_t[:], EPS)
            lnsx2_t = const.tile([P, 1], F32)
            nc.vector.memset(lnsx2_t[:], LN_SX2)
            bkv = const.tile([1, 2 * C], BF16)
            bproj = const.tile([1, C], BF16)
            if has_bkv:
                nc.sync.dma_start(out=bkv[:], in_=bkv_d[:])
            if has_bproj:
                nc.sync.dma_start(out=bproj[:], in_=bproj_d[:])

            # ---- helpers ----
            def ln_rstd(var_ap, rs_ap, nsub, logmul_ap, tag):
                # rs = exp(-0.5*ln(var+eps) + logmul)  == exp(logmul)/sqrt(var+eps)
                lnv = stats.tile([P, nsub], F32, tag=f"lnv{tag}")
                nc.scalar.activation(out=lnv[:], in_=var_ap, func=AF.Ln, bias=eps_t[:])
                if logmul_ap is None:
                    nc.scalar.activation(
                        out=rs_ap, in_=lnv[:], func=AF.Exp, scale=-0.5
                    )
                else:
                    nc.scalar.activation(
                        out=rs_ap, in_=lnv[:], func=AF.Exp, scale=-0.5,
                        bias=logmul_ap,
                    )

            def ln_normalize(src_ap, dst, mv, rs):
                nc.vector.tensor_scalar(
                    out=dst,
                    in0=src_ap,
                    scalar1=mv[:, 0:1],
                    scalar2=rs,
                    op0=ALU.subtract,
                    op1=ALU.mult,
                )

            # =============== Phase A: LN1, xhat^T, kv, attn accum, q^T =======
            ab_ctx = contextlib.ExitStack()
            ps_attn = ab_ctx.enter_context(
                tc.tile_pool(name="ps_attn", bufs=1, space="PSUM")
            )
            attn_ps = [
                ps_attn.tile([P, C], F32, name=f"attn_ps{i}") for i in range(2)
            ]
            with tc.tile_pool(name="ps_kv", bufs=2, space="PSUM") as ps_kv, \
                 tc.tile_pool(name="ps_q", bufs=1, space="PSUM") as ps_q:

                def q_pass(p):
                    # q^T for token chunks 4p..4p+3 (feature-major, fp8 out)
                    for fc in range(2):
                        qps = [
                            ps_q.tile([P, 512], F32, tag=f"q{j}", name=f"qp{p}{fc}{j}")
                            for j in range(4)
                        ]
                        for kc in range(2):
                            for j in range(4):
                                ch = 4 * p + j
                                nc.tensor.matmul(
                                    qps[j][:],
                                    wq[:, kc, fc * P : (fc + 1) * P],
                                    xhT[:, kc, ch * 512 : (ch + 1) * 512],
                                    start=(kc == 0),
                                    stop=(kc == 1),
                                )
                        for j in range(4):
                            ch = 4 * p + j
                            # qT8 = psum + bq_scaled   (cast fp8)
                            nc.scalar.activation(
                                out=qT8[:, fc, ch * 512 : (ch + 1) * 512],
                                in_=qps[j][:],
                                func=AF.Identity,
                                bias=bq[:, fc : fc + 1],
                            )

                for g in range(NG):
                    idxs = [4 * g + s for s in range(4)]
                    mv4 = stats.tile([P, 4, 2], F32, tag="mv")
                    rs4 = stats.tile([P, 4], F32, tag="rs")
                    for s, i in enumerate(idxs):
                        st = stats.tile([P, 6], F32, tag="bn")
                        nc.vector.bn_stats(out=st[:], in_=x_sb[:, i, :])
                        nc.vector.bn_aggr(out=mv4[:, s, :], in_=st[:])
                    ln_rstd(mv4[:, :, 1], rs4[:], 4, None, "a")
                    for s, i in enumerate(idxs):
                        xhat = work.tile([P, C], BF16, tag="xhat")
                        ln_normalize(
                            x_sb[:, i, :], xhat[:], mv4[:, s, :], rs4[:, s : s + 1]
                        )
                        for c in range(2):
                            nc.sync.dma_start_transpose(
                                out=xhT[:, c, i * P : (i + 1) * P],
                                in_=xhat[:, c * P : (c + 1) * P],
                            )
                    # kv (token-major) + attn accumulation per pair
                    for par in range(2):
                        k2 = kvp.tile([P, 2, C], FP8, tag="k2")
                        v2 = kvp.tile([P, 2, C], FP8, tag="v2")
                        for u in range(2):
                            i = idxs[2 * par + u]
                            kv_ps = ps_kv.tile([P, 512], F32, tag="kv")
                            nc.tensor.matmul(
                                kv_ps[:],
                                xhT[:, 0, i * P : (i + 1) * P],
                                wkv[:, 0, :],
                                start=True,
                                stop=False,
                            )
                            nc.tensor.matmul(
                                kv_ps[:],
                                xhT[:, 1, i * P : (i + 1) * P],
                                wkv[:, 1, :],
                                start=False,
                                stop=not has_bkv,
                            )
                            if has_bkv:
                                nc.tensor.matmul(
                                    kv_ps[:],
                                    ones_row[:],
                                    bkv[:],
                                    start=False,
                                    stop=True,
                                )
                            nc.vector.tensor_copy(
                                out=k2[:, u, :], in_=kv_ps[:, 0:C]
                            )
                            nc.scalar.copy(
                                out=v2[:, u, :], in_=kv_ps[:, C : 2 * C]
                            )
                        pair = 2 * g + par
                        for half in range(2):
                            nc.tensor.matmul(
                                attn_ps[half][:],
                                k2[:, :, half * P : (half + 1) * P],
                                v2[:, :, :],
                                start=(pair == 0),
                                stop=(pair == 2 * NG - 1),
                                perf_mode=DR,
                            )
                    if g == 3:
                        q_pass(0)
                    elif g == 7:
                        q_pass(1)

            # =============== Phase B: softmax -> E (fused attn@Wproj) ========
            BdT = const.tile([P, 2, P], BF16)
            nc.vector.memset(BdT[:], 0.0)
            E8 = const.tile([P, 2, C], FP8)
            with tc.tile_pool(name="ps_e", bufs=2, space="PSUM") as ps_e:
                for half in range(2):
                    a_sb = work.tile([P, HD], F32, tag="attn")
                    for h in range(4):
                        hh = half * 4 + h
                        nc.vector.tensor_scalar(
                            out=a_sb[h * HD : (h + 1) * HD, :],
                            in0=attn_ps[half][
                                h * HD : (h + 1) * HD, hh * HD : (hh + 1) * HD
                            ],
                            scalar1=INV_ATTN,
                            scalar2=None,
                            op0=ALU.mult,
                        )
                    negmax = stats.tile([P, 1], F32, tag="negmax")
                    nc.vector.tensor_reduce(
                        out=negmax[:], in_=a_sb[:], axis=AX.X, op=ALU.max, negate=True
                    )
                    exps = work.tile([P, HD], F32, tag="exps")
                    nc.scalar.activation(
                        out=exps[:], in_=a_sb[:], func=AF.Exp, bias=negmax[:]
                    )
                    ssum = stats.tile([P, 1], F32, tag="ssum")
                    nc.vector.tensor_reduce(
                        out=ssum[:], in_=exps[:], axis=AX.X, op=ALU.add
                    )
                    rec = stats.tile([P, 1], F32, tag="rec")
                    nc.vector.reciprocal(out=rec[:], in_=ssum[:])
                    for h in range(4):
                        sl = slice(h * HD, (h + 1) * HD)
                        nc.vector.tensor_scalar(
                            out=BdT[sl, half, sl],
                            in0=exps[sl, :],
                            scalar1=rec[sl, 0:1],
                            scalar2=None,
                            op0=ALU.mult,
                        )
                for half in range(2):
                    e_ps = ps_e.tile([P, C], F32, tag="e")
                    nc.tensor.matmul(
                        e_ps[:],
                        BdT[:, half, :],
                        wproj[:, half, :],
                        start=True,
                        stop=True,
                    )
                    nc.vector.tensor_scalar(
                        out=E8[:, half, :],
                        in0=e_ps[:],
                        scalar1=S_E,
                        scalar2=None,
                        op0=ALU.mult,
                    )
            ab_ctx.close()  # free attn psum banks before phase C pools open

            # =============== Phase C: proj+res+LN2 / fc1+gelu / fc2 ==========
            # Pipelined in two half-N passes so C1/LN2 of half 1 overlaps the
            # ACT-bound gelu window of half 0, and fc2/outputs of half 0
            # overlap the gelu window of half 1.
            ps_c = ctx.enter_context(tc.tile_pool(name="ps_c", bufs=2, space="PSUM"))
            ps_f = ctx.enter_context(tc.tile_pool(name="ps_f", bufs=1, space="PSUM"))
            ps_m = ctx.enter_context(tc.tile_pool(name="ps_m", bufs=1, space="PSUM"))

            def c1_tile(i):
                p_ps = ps_c.tile([P, C], F32, tag="c1", name=f"pp{i}")
                nc.tensor.matmul(
                    p_ps[:],
                    qT8[:, :, i * P : (i + 1) * P],
                    E8[:, :, :],
                    start=True,
                    stop=not has_bproj,
                    perf_mode=DR,
                )
                if has_bproj:
                    nc.tensor.matmul(
                        p_ps[:], ones_row[:], bproj[:], start=False, stop=True
                    )
                # h1 = x + proj_out  (f32, token-major)
                nc.vector.scalar_tensor_tensor(
                    out=h1_sb[:, i, :],
                    in0=p_ps[:],
                    scalar=INV_PROJ,
                    in1=x_sb[:, i, :],
                    op0=ALU.mult,
                    op1=ALU.add,
                )
                st = stats.tile([P, 6], F32, tag="bn", name=f"st{i}")
                nc.vector.bn_stats(out=st[:], in_=h1_sb[:, i, :])
                nc.vector.bn_aggr(out=mv32[:, i, :], in_=st[:])

            def ln2_group(g):
                for s in range(4):
                    i = 4 * g + s
                    x2 = work.tile([P, C], BF16, tag="x2")
                    ln_normalize(
                        h1_sb[:, i, :], x2[:], mv32[:, i, :], rs32[:, i : i + 1]
                    )
                    for c in range(2):
                        nc.sync.dma_start_transpose(
                            out=x2T[:, c, i * P : (i + 1) * P],
                            in_=x2[:, c * P : (c + 1) * P],
                        )
                (nc.vector if os.environ.get("BASS_DVECAST", "0") == "1"
                 else nc.gpsimd).tensor_copy(
                    out=x2T8[:, :, g * 512 : (g + 1) * 512],
                    in_=x2T[:, :, g * 512 : (g + 1) * 512],
                )

            def fc1_half(hf):
                # hidden rows, fp8 DoubleRow; 2 token-quarters per half
                for hc in range(8):
                    for tq in range(2):
                        q0 = (2 * hf + tq) * 1024
                        f_ps = ps_f.tile([P, 1024], F32, tag="f")
                        for u in range(2):
                            nc.tensor.matmul(
                                f_ps[:, u * 512 : (u + 1) * 512],
                                w18[:, :, hc * P : (hc + 1) * P],
                                x2T8[:, :, q0 + u * 512 : q0 + (u + 1) * 512],
                                start=True,
                                stop=True,
                                perf_mode=DR,
                            )
                        for u in range(2):
                            nc.scalar.activation(
                                out=g1T8[:, hc, q0 + u * 512 : q0 + (u + 1) * 512],
                                in_=f_ps[:, u * 512 : (u + 1) * 512],
                                func=AF.Gelu if os.environ.get('BASS_NOGELU','0')=='0' else AF.Identity,
                                bias=b1[:, hc : hc + 1],
                                scale=1.0 / (S_X2 * s_w1),
                            )

            def fc2_half(hf):
                # feature-major: stationary = w2 pair slices (8 ldweights)
                for cs in range(2):
                    mps = [
                        ps_m.tile([P, 512], F32, tag=f"m{j}", name=f"mp{hf}{cs}{j}")
                        for j in range(4)
                    ]
                    for j in range(4):
                        for tch in range(4):
                            t0 = (4 * hf + tch) * 512
                            nc.tensor.matmul(
                                mps[tch][:],
                                w28[:, 2 * j : 2 * j + 2, cs * P : (cs + 1) * P],
                                g1T8[:, 2 * j : 2 * j + 2, t0 : t0 + 512],
                                start=(j == 0),
                                stop=(j == 3),
                                perf_mode=DR,
                            )
                    for tch in range(4):
                        t0 = (4 * hf + tch) * 512
                        nc.vector.tensor_scalar(
                            out=mTb[:, cs, t0 : t0 + 512],
                            in0=mps[tch][:],
                            scalar1=1.0 / s_w2,
                            scalar2=bfc2[:, cs : cs + 1],
                            op0=ALU.mult,
                            op1=ALU.add,
                        )

            def out_half(hf):
                for g in range(4 * hf, 4 * hf + 4):
                    och = outp.tile([P, 4, C], F32, tag="oc")
                    for s in range(4):
                        i = 4 * g + s
                        m_tok = work.tile([P, C], BF16, tag="mtok")
                        for c in range(2):
                            nc.sync.dma_start_transpose(
                                out=m_tok[:, c * P : (c + 1) * P],
                                in_=mTb[:, c, i * P : (i + 1) * P],
                            )
                        t1 = outp.tile([P, C], F32, tag="t1")
                        nc.vector.tensor_tensor(
                            out=t1[:], in0=m_tok[:], in1=h1_sb[:, i, :], op=ALU.add
                        )
                        nc.gpsimd.tensor_tensor(
                            out=och[:, s, :], in0=t1[:], in1=x_sb[:, i, :], op=ALU.add
                        )
                    nc.sync.dma_start(
                        out=out_d[512 * g : 512 * (g + 1), :].rearrange(
                            "(s p) c -> p s c", p=P
                        ),
                        in_=och[:],
                    )

            def half_rstd(hf):
                sl = slice(16 * hf, 16 * (hf + 1))
                ln_rstd(mv32[:, sl, 1], rs32[:, sl], 16, lnsx2_t[:], f"c{hf}")

            if os.environ.get("BASS_SERIAL", "0") == "1":
                for i in range(32):
                    c1_tile(i)
                half_rstd(0)
                half_rstd(1)
                for g in range(8):
                    ln2_group(g)
                fc1_half(0)
                fc1_half(1)
                fc2_half(0)
                fc2_half(1)
                out_half(0)
                out_half(1)
            else:
                # half 0
                for i in range(16):
                    c1_tile(i)
                half_rstd(0)
                for g in range(4):
                    ln2_group(g)
                fc1_half(0)
                # half 1 (overlaps gelu window of half 0)
                for i in range(16, 32):
                    c1_tile(i)
                half_rstd(1)
                for g in range(4, 8):
                    ln2_group(g)
                fc2_half(0)
                out_half(0)
                fc1_half(1)
                fc2_half(1)
                out_half(1)

            if DBG:
                for k, src in {
                    "xhT": xhT,
                    "qT": qT8,
                    "E": E8,
                    "h1": h1_sb,
                    "x2T": x2T8,
                    "g1T": g1T8,
                    "mT": mTb,
                    "w18": w18,
                    "w28": w28,
                }.items():
                    nc.sync.dma_start(
                        out=dbg_d[k][:], in_=src[:].rearrange("p a b -> p (a b)")
                    )

    _split_sync_waits(nc)
    return nc


_CACHE = {}


def _get_nc(key):
    if key not in _CACHE:
        _CACHE[key] = _build_nc(*key)
    return _CACHE[key]


def _pow2_floor(x):
    return float(2.0 ** np.floor(np.log2(x)))


def _prep_inputs(inputs):
    f32 = lambda k: np.asarray(inputs[k], dtype=np.float32)
    qkv_w, qkv_b = f32("qkv_w"), f32("qkv_b")
    proj_w, proj_b = f32("proj_w"), f32("proj_b")
    ln1_g, ln1_b = f32("ln1_g"), f32("ln1_b")
    ln2_g, ln2_b = f32("ln2_g"), f32("ln2_b")
    fc1_w, fc1_b = f32("fc1_w"), f32("fc1_b")
    fc2_w, fc2_b = f32("fc2_w"), f32("fc2_b")

    scale = HD ** (-0.5)

    # Fold LN1 affine into qkv: LN1(x)@W+b = xhat@(g*W) + (ln1_b@W + b)
    wqkv_f = ln1_g[:, None] * qkv_w
    bqkv_f = ln1_b @ qkv_w + qkv_b
    # Fold channel-attention scale into k, then fp8 range scales into k/v/q
    wk = wqkv_f[:, C : 2 * C] * (scale * S_K)
    wv = wqkv_f[:, 2 * C : 3 * C] * S_V
    wqs = wqkv_f[:, 0:C] * S_Q
    bk = bqkv_f[C : 2 * C] * (scale * S_K)
    bv = bqkv_f[2 * C : 3 * C] * S_V
    bqs = bqkv_f[0:C] * S_Q
    # Fold LN2 affine into fc1
    w1_f = ln2_g[:, None] * fc1_w
    b1_f = ln2_b @ fc1_w + fc1_b

    s_w1 = _pow2_floor(224.0 / max(np.abs(w1_f).max(), 1e-30))
    s_w2 = _pow2_floor(224.0 / max(np.abs(fc2_w).max(), 1e-30))

    wkv = np.concatenate([wk, wv], axis=1)  # [256, 512]
    bkv = np.concatenate([bk, bv])

    has_bkv = bool(np.any(bkv != 0))
    has_bproj = bool(np.any(proj_b != 0))

    shared = {
        "wkv": wkv.reshape(2, P, 2 * C).astype(NP_BF16),
        "wq": wqs.reshape(2, P, C).astype(NP_BF16),
        "wproj": proj_w.reshape(2, P, C).astype(NP_BF16),
        "w1": (w1_f * s_w1).reshape(2, P, HID).astype(NP_FP8),
        "w2": (fc2_w * s_w2).reshape(8, P, C).astype(NP_FP8),
        "bq": bqs.reshape(2, P, 1).astype(np.float32),
        "b1": b1_f.reshape(8, P, 1).astype(np.float32),
        "bfc2": fc2_b.reshape(2, P, 1).astype(np.float32),
        "bkv": bkv.reshape(1, 2 * C).astype(NP_BF16),
        "bproj": proj_b.reshape(1, C).astype(NP_BF16),
    }
    return shared, (has_bkv, has_bproj, s_w1, s_w2)


def kernel(x, **weights):
    x = np.asarray(x, dtype=np.float32)
    shared, key = _prep_inputs(weights)
    nc = _get_nc(key)
    in_maps = [dict(shared, x=np.ascontiguousarray(x[b])) for b in range(B)]
    res = run_bass_kernel_spmd(nc, in_maps, list(range(B)))
    out = np.stack([res.results[b]["out"] for b in range(B)], axis=0)
    return out.astype(np.float32)


# revision 21
# speedup vs baseline: 1.4897x; 1.4897x over previous
"""ChannelBlock (dense transformer block with channel/cross-covariance attention)
Trainium2 Bass kernel, data-parallel over batch across 8 NeuronCores.

Contract: kernel(**inputs) takes FULL unsharded inputs (np arrays), returns the
FULL output [8, 4096, 256] float32.

v2.1 design notes (per-core, one batch element):
 - phase A (LN1 + kv + q) runs bf16 with k/v/q range scales folded into the
   weights host-side, so psum evictions are pure casts.
 - channel-attention accumulation (k^T v), proj, fc1 and fc2 run fp8e4
   DoubleRow (2 fp8 weights per PE cell).
 - activation transposes use PE transpose-mode (bf16 in -> bf16 psum), so
   their evictions can run in the DVE 2x mode.
 - fc2 is computed feature-major (stationary = w2 pair slices, reused across
   all tokens: 8 ldweights) and transposed back via the PE.
 - LN rstd = exp(-0.5*ln(var+eps)): ln+exp share one ACT table set, so the
   only ACT table switch in the kernel is to gelu for the MLP.
"""

import os

import numpy as np

import concourse.bass as bass
import concourse.bass_utils as _bu
import concourse.tile as tile
from concourse import masks, mybir
from concourse.bass_utils import run_bass_kernel_spmd
from concourse.vector_clock import ScopedClock
import bass_rust

# Optionally re-enable walrus' LDWEIGHTS optimization (off by default in this
# container's compile driver); gated so it can be A/B tested.
if os.environ.get("BASS_LDW_OPT", "0") == "1" and not getattr(
    _bu, "_ldw_patched", False
):
    _orig_run_command = _bu.run_command

    def _run_command_ldw(cmd, **kw):
        if isinstance(cmd, list):
            cmd = [
                "--enable-ldw-opt=true" if c == "--enable-ldw-opt=false" else c
                for c in cmd
            ]
        return _orig_run_command(cmd, **kw)

    _bu.run_command = _run_command_ldw
    _bu._ldw_patched = True

# ----------------------------------------------------------------------------
# Workaround: this container's walrus (CoreV3) only supports ONE sync-wait
# command on TPB_CTRL instructions (Drain).  Tile's kernel-tail drain piles all
# outstanding proc waits onto a single Drain -> split into a chain of Drains
# with one wait each.
# ----------------------------------------------------------------------------
_MAX_DRAIN_WAITS = 1


def _patched_drain_and_barrier(self, tick_clock, wait_clock):
    drain_inst = self.nc.sync.drain()
    wait_clock.add_sem_waits(
        drain_inst.ins, ScopedClock({None: tick_clock.global_clock})
    )
    mi = drain_inst.ins
    si = mi.sync_info
    waits = list(si.on_wait) if si else []
    if len(waits) > _MAX_DRAIN_WAITS:
        mi.sync_info = bass_rust.SyncInfo(
            on_wait=waits[:_MAX_DRAIN_WAITS], on_update=list(si.on_update)
        )
        for i in range(_MAX_DRAIN_WAITS, len(waits), _MAX_DRAIN_WAITS):
            extra = self.nc.sync.drain()
            extra.ins.sync_info = bass_rust.SyncInfo(
                on_wait=waits[i : i + _MAX_DRAIN_WAITS], on_update=[]
            )
    self.nc.all_engine_barrier()
    popped = self.nc._tile_sem_poison_stack.pop()
    assert popped is self._sem_poison
    self.nc.clear_and_free_semaphores(list(self.sems.allocated().values()))
    self.nc.all_engine_barrier()


tile.TileContext._drain_and_barrier = _patched_drain_and_barrier

_nop_counter = [0]


def _split_sync_waits(nc, cap=1):
    """Walrus in this container rejects instructions with more than `cap`
    sync-wait commands.  Hoist excess waits onto same-engine NOPs inserted
    immediately before the instruction (engine streams are in-order, so the
    semantics are unchanged)."""
    for f in nc.m.functions:
        for blk in f.blocks:
            changed = False
            new = []
            for inst in blk.instructions:
                si = inst.sync_info
                waits = list(si.on_wait) if si is not None else []
                # ldw-opt rejects Ldweights carrying sync waits; hoist them.
                is_ldw = inst.__class__.__name__ == "InstLdweights"
                eff_cap = 0 if (is_ldw and waits) else cap
                if len(waits) > eff_cap:
                    if is_ldw:
                        excess, keep = waits, []
                    else:
                        excess, keep = waits[:-cap], waits[-cap:]
                    for j in range(0, len(excess), cap):
                        _nop_counter[0] += 1
                        nop = mybir.InstNoOp(
                            name=f"NW-{_nop_counter[0]}", ins=[], outs=[]
                        )
                        nop.engine = inst.engine
                        nop.sync_info = bass_rust.SyncInfo(
                            on_wait=excess[j : j + cap], on_update=[]
                        )
                        new.append(nop)
                    inst.sync_info = bass_rust.SyncInfo(
                        on_wait=keep, on_update=list(si.on_update)
                    )
                    changed = True
                new.append(inst)
            if changed:
                blk.instructions = new


# ----------------------------------------------------------------------------
# Problem constants (hardcoded per the task contract)
# ----------------------------------------------------------------------------
B = 8
N = 4096
C = 256
H = 8
HD = C // H  # 32
HID = 1024
EPS = 1e-5
P = 128
NTILES = N // P  # 32
NG = NTILES // 4  # 8 groups of 4 tiles (512 tokens each)

F32 = mybir.dt.float32
BF16 = mybir.dt.bfloat16
FP8 = mybir.dt.float8e4
NP_BF16 = mybir.dt.np(BF16)
NP_FP8 = mybir.dt.np(FP8)
FP8_SAFE = 224.0  # ml_dtypes float8_e4m3 max finite is 240

AF = mybir.ActivationFunctionType
ALU = mybir.AluOpType
AX = mybir.AxisListType
DR = mybir.MatmulPerfMode.DoubleRow

# activation scales (power-of-two; fp8 range management only)
S_K = 64.0
S_V = 16.0
S_Q = 16.0
S_X2 = 16.0
S_E = 256.0
INV_ATTN = 1.0 / (S_K * S_V)
INV_PROJ = 1.0 / (S_Q * S_E)
LN_SX2 = float(np.log(S_X2))


def _build_nc(has_bkv, has_bproj, s_w1, s_w2):
    nc = bass.Bass()

    # ---- DRAM I/O ----
    x_d = nc.declare_dram_parameter("x", [N, C], F32, isOutput=False)
    wkv_d = nc.declare_dram_parameter("wkv", [2, P, 2 * C], BF16, isOutput=False)
    wq_d = nc.declare_dram_parameter("wq", [2, P, C], BF16, isOutput=False)
    wproj_d = nc.declare_dram_parameter("wproj", [2, P, C], BF16, isOutput=False)
    w1_d = nc.declare_dram_parameter("w1", [2, P, HID], FP8, isOutput=False)
    w2_d = nc.declare_dram_parameter("w2", [8, P, C], FP8, isOutput=False)
    bq_d = nc.declare_dram_parameter("bq", [2, P, 1], F32, isOutput=False)
    b1_d = nc.declare_dram_parameter("b1", [8, P, 1], F32, isOutput=False)
    bfc2_d = nc.declare_dram_parameter("bfc2", [2, P, 1], F32, isOutput=False)
    bkv_d = nc.declare_dram_parameter("bkv", [1, 2 * C], BF16, isOutput=False)
    bproj_d = nc.declare_dram_parameter("bproj", [1, C], BF16, isOutput=False)
    out_d = nc.declare_dram_parameter("out", [N, C], F32, isOutput=True)
    DBG = os.environ.get("BASS_DBG", "0") == "1"
    if DBG:
        dbg_d = {
            k: nc.declare_dram_parameter(f"dbg_{k}", shp, dt, isOutput=True)
            for k, (shp, dt) in {
                "xhT": ([P, 2 * N], BF16),
                "qT": ([P, 2 * N], FP8),
                "E": ([P, 2 * C], FP8),
                "h1": ([P, NTILES * C], F32),
                "x2T": ([P, 2 * N], FP8),
                "g1T": ([P, 8 * N], FP8),
                "mT": ([P, 2 * N], BF16),
            }.items()
        }

    with tile.TileContext(nc) as tc:
        import contextlib

        ctx = contextlib.ExitStack()
        with ctx:
            const = ctx.enter_context(tc.tile_pool(name="const", bufs=1))
            xres = ctx.enter_context(tc.tile_pool(name="xres", bufs=1))
            stats = ctx.enter_context(tc.tile_pool(name="stats", bufs=4))
            work = ctx.enter_context(tc.tile_pool(name="work", bufs=4))
            kvp = ctx.enter_context(tc.tile_pool(name="kvp", bufs=3))
            outp = ctx.enter_context(tc.tile_pool(name="outp", bufs=3))
            # transpose + c1 psum pool, shared across phases A and C
            ps_t = ctx.enter_context(tc.tile_pool(name="ps_t", bufs=1, space="PSUM"))

            # ---- residents ----
            x_sb = xres.tile([P, NTILES, C], F32)  # raw x, token-major
            h1_sb = xres.tile([P, NTILES, C], F32)  # x + attn, token-major
            xhT = xres.tile([P, 2, N], BF16)  # LN1(x)^T  (feature-major)
            qT8 = xres.tile([P, 2, N], FP8)  # (q*S_Q)^T
            x2T8 = xres.tile([P, 2, N], FP8)  # (LN2(h1)*S_X2)^T
            g1T8 = xres.tile([P, 8, N], FP8)  # gelu(fc1)^T
            mTb = xres.tile([P, 2, N], BF16)  # fc2 out, feature-major
            mv32 = xres.tile([P, NTILES, 2], F32)
            rs32 = xres.tile([P, NTILES], F32)

            # ---- input DMAs: x first (compute starts on it), weights after --
            for g in range(NG):
                nc.sync.dma_start(
                    out=x_sb[:, 4 * g : 4 * g + 4, :],
                    in_=x_d[512 * g : 512 * (g + 1), :].rearrange(
                        "(s p) c -> p s c", p=P
                    ),
                )
            wkv = const.tile([P, 2, 2 * C], BF16)
            wq = const.tile([P, 2, C], BF16)
            wproj = const.tile([P, 2, C], BF16)
            bq = const.tile([P, 2], F32)
            nc.sync.dma_start(out=wkv[:], in_=wkv_d.rearrange("c p f -> p c f"))
            nc.sync.dma_start(out=wq[:], in_=wq_d.rearrange("c p f -> p c f"))
            nc.sync.dma_start(out=wproj[:], in_=wproj_d.rearrange("c p f -> p c f"))
            for c in range(2):
                nc.sync.dma_start(out=bq[:, c : c + 1], in_=bq_d[c])
            w18 = const.tile([P, 2, HID], FP8)
            w28 = const.tile([P, 8, C], FP8)
            b1 = const.tile([P, 8], F32)
            bfc2 = const.tile([P, 2], F32)
            nc.sync.dma_start(out=w18[:], in_=w1_d.rearrange("c p f -> p c f"))
            nc.sync.dma_start(out=w28[:], in_=w2_d.rearrange("c p f -> p c f"))
            for c in range(8):
                nc.sync.dma_start(out=b1[:, c : c + 1], in_=b1_d[c])
            for c in range(2):
                nc.sync.dma_start(out=bfc2[:, c : c + 1], in_=bfc2_d[c])
            ident = const.tile([P, P], BF16)
            masks.make_identity(nc, ident[:])
            ones_row = const.tile([1, P], BF16)
            nc.vector.memset(ones_row[:], 1.0)
            eps_t = const.tile([P, 1], F32)
            nc.vector.memset(eps_t[:], EPS)
            lnsx2_t = const.tile([P, 1], F32)
            nc.vector.memset(lnsx2_t[:], LN_SX2)
            bkv = const.tile([1, 2 * C], BF16)
            bproj = const.tile([1, C], BF16)
            if has_bkv:
                nc.sync.dma_start(out=bkv[:], in_=bkv_d[:])
            if has_bproj:
                nc.sync.dma_start(out=bproj[:], in_=bproj_d[:])

            # ---- helpers ----
            def ln_rstd(var_ap, rs_ap, nsub, logmul_ap, tag):
                # rs = exp(-0.5*ln(var+eps) + logmul) == exp(logmul)/sqrt(var+eps)
                lnv = stats.tile([P, nsub], F32, tag=f"lnv{tag}")
                nc.scalar.activation(
                    out=lnv[:], in_=var_ap, func=AF.Ln, bias=eps_t[:]
                )
                if logmul_ap is None:
                    nc.scalar.activation(
                        out=rs_ap, in_=lnv[:], func=AF.Exp, scale=-0.5
                    )
                else:
                    nc.scalar.activation(
                        out=rs_ap, in_=lnv[:], func=AF.Exp, scale=-0.5,
                        bias=logmul_ap,
                    )

            def ln_normalize(src_ap, dst, mv, rs, eng):
                eng.tensor_scalar(
                    out=dst,
                    in0=src_ap,
                    scalar1=mv[:, 0:1],
                    scalar2=rs,
                    op0=ALU.subtract,
                    op1=ALU.mult,
                )

            def pe_transpose_to(dst3_ap, src_sb, s):
                # dst3_ap: [P, 2, 128] slice of a feature-major resident;
                # src_sb: [P, 256] bf16 token-major tile
                tp = ps_t.tile([P, C], BF16, tag="tp")
                for c in range(2):
                    nc.tensor.transpose(
                        tp[:, c * P : (c + 1) * P],
                        src_sb[:, c * P : (c + 1) * P],
                        ident[:],
                    )
                src3 = tp[:].rearrange("p (c t) -> p c t", c=2)
                if s % 2 == 0:
                    nc.vector.tensor_copy(out=dst3_ap, in_=src3)
                else:
                    nc.scalar.copy(out=dst3_ap, in_=src3)

            # =============== Phase A: LN1, xhat^T, kv, attn accum, q^T =======
            ab_ctx = contextlib.ExitStack()
            ps_attn = ab_ctx.enter_context(
                tc.tile_pool(name="ps_attn", bufs=1, space="PSUM")
            )
            attn_ps = [
                ps_attn.tile([P, C], F32, name=f"attn_ps{i}") for i in range(2)
            ]
            with tc.tile_pool(name="ps_kv", bufs=2, space="PSUM") as ps_kv, \
                 tc.tile_pool(name="ps_q", bufs=2, space="PSUM") as ps_q:

                def q_pass(p):
                    # q^T for token chunks 4p..4p+3 (feature-major, fp8 out)
                    for fc in range(2):
                        for j in range(4):
                            ch = 4 * p + j
                            qps = ps_q.tile([P, 512], F32, tag="q")
                            for kc in range(2):
                                nc.tensor.matmul(
                                    qps[:],
                                    wq[:, kc, fc * P : (fc + 1) * P],
                                    xhT[:, kc, ch * 512 : (ch + 1) * 512],
                                    start=(kc == 0),
                                    stop=(kc == 1),
                                )
                            # qT8 = psum + bq_scaled   (cast fp8)
                            nc.scalar.activation(
                                out=qT8[:, fc, ch * 512 : (ch + 1) * 512],
                                in_=qps[:],
                                func=AF.Identity,
                                bias=bq[:, fc : fc + 1],
                            )

                for g in range(NG):
                    idxs = [4 * g + s for s in range(4)]
                    mv4 = stats.tile([P, 4, 2], F32, tag="mv")
                    rs4 = stats.tile([P, 4], F32, tag="rs")
                    for s, i in enumerate(idxs):
                        st = stats.tile([P, 6], F32, tag="bn")
                        nc.vector.bn_stats(out=st[:], in_=x_sb[:, i, :])
                        nc.vector.bn_aggr(out=mv4[:, s, :], in_=st[:])
                    ln_rstd(mv4[:, :, 1], rs4[:], 4, None, "a")
                    for s, i in enumerate(idxs):
                        xhat = work.tile([P, C], BF16, tag="xhat")
                        ln_normalize(
                            x_sb[:, i, :],
                            xhat[:],
                            mv4[:, s, :],
                            rs4[:, s : s + 1],
                            nc.vector if s % 2 == 0 else nc.gpsimd,
                        )
                        pe_transpose_to(xhT[:, :, i * P : (i + 1) * P], xhat[:], s)
                    # kv (token-major) + attn accumulation per pair
                    for par in range(2):
                        kv8 = kvp.tile([P, 2, 512], FP8, tag="kv8")
                        for u in range(2):
                            i = idxs[2 * par + u]
                            kv_ps = ps_kv.tile([P, 512], F32, tag="kv")
                            nc.tensor.matmul(
                                kv_ps[:],
                                xhT[:, 0, i * P : (i + 1) * P],
                                wkv[:, 0, :],
                                start=True,
                                stop=False,
                            )
                            nc.tensor.matmul(
                                kv_ps[:],
                                xhT[:, 1, i * P : (i + 1) * P],
                                wkv[:, 1, :],
                                start=False,
                                stop=not has_bkv,
                            )
                            if has_bkv:
                                nc.tensor.matmul(
                                    kv_ps[:],
                                    ones_row[:],
                                    bkv[:],
                                    start=False,
                                    stop=True,
                                )
                            if u == 0:
                                nc.vector.tensor_copy(
                                    out=kv8[:, u, :], in_=kv_ps[:]
                                )
                            else:
                                nc.scalar.copy(out=kv8[:, u, :], in_=kv_ps[:])
                        pair = 2 * g + par
                        for half in range(2):
                            nc.tensor.matmul(
                                attn_ps[half][:],
                                kv8[:, :, half * P : (half + 1) * P],
                                kv8[:, :, C : 2 * C],
                                start=(pair == 0),
                                stop=(pair == 2 * NG - 1),
                                perf_mode=DR,
                            )
                    if g == 3:
                        q_pass(0)
                    elif g == 7:
                        q_pass(1)

            # =============== Phase B: softmax -> E (fused attn@Wproj) ========
            BdT = const.tile([P, 2, P], BF16)
            nc.vector.memset(BdT[:], 0.0)
            E8 = const.tile([P, 2, C], FP8)
            with tc.tile_pool(name="ps_e", bufs=2, space="PSUM") as ps_e:
                for half in range(2):
                    a_sb = work.tile([P, HD], F32, tag="attn")
                    for h in range(4):
                        hh = half * 4 + h
                        nc.vector.tensor_scalar(
                            out=a_sb[h * HD : (h + 1) * HD, :],
                            in0=attn_ps[half][
                                h * HD : (h + 1) * HD, hh * HD : (hh + 1) * HD
                            ],
                            scalar1=INV_ATTN,
                            scalar2=None,
                            op0=ALU.mult,
                        )
                    negmax = stats.tile([P, 1], F32, tag="negmax")
                    nc.vector.tensor_reduce(
                        out=negmax[:], in_=a_sb[:], axis=AX.X, op=ALU.max,
                        negate=True,
                    )
                    exps = work.tile([P, HD], F32, tag="exps")
                    nc.scalar.activation(
                        out=exps[:], in_=a_sb[:], func=AF.Exp, bias=negmax[:]
                    )
                    ssum = stats.tile([P, 1], F32, tag="ssum")
                    nc.vector.tensor_reduce(
                        out=ssum[:], in_=exps[:], axis=AX.X, op=ALU.add
                    )
                    rec = stats.tile([P, 1], F32, tag="rec")
                    nc.vector.reciprocal(out=rec[:], in_=ssum[:])
                    for h in range(4):
                        sl = slice(h * HD, (h + 1) * HD)
                        nc.vector.tensor_scalar(
                            out=BdT[sl, half, sl],
                            in0=exps[sl, :],
                            scalar1=rec[sl, 0:1],
                            scalar2=None,
                            op0=ALU.mult,
                        )
                for half in range(2):
                    e_ps = ps_e.tile([P, C], F32, tag="e")
                    nc.tensor.matmul(
                        e_ps[:],
                        BdT[:, half, :],
                        wproj[:, half, :],
                        start=True,
                        stop=True,
                    )
                    nc.vector.tensor_scalar(
                        out=E8[:, half, :],
                        in0=e_ps[:],
                        scalar1=S_E,
                        scalar2=None,
                        op0=ALU.mult,
                    )
            ab_ctx.close()  # free attn psum banks before phase C pools open

            # =============== Phase C: proj+res+LN2 / fc1+gelu / fc2 ==========
            # Pipelined in two half-N passes so C1/LN2 of half 1 overlaps the
            # ACT-bound gelu window of half 0, and fc2/outputs of half 0
            # overlap the gelu window of half 1.
            ps_f = ctx.enter_context(tc.tile_pool(name="ps_f", bufs=1, space="PSUM"))
            ps_m = ctx.enter_context(tc.tile_pool(name="ps_m", bufs=1, space="PSUM"))

            def c1_tile(i):
                p_ps = ps_t.tile([P, C], F32, tag="c1", name=f"pp{i}")
                nc.tensor.matmul(
                    p_ps[:],
                    qT8[:, :, i * P : (i + 1) * P],
                    E8[:, :, :],
                    start=True,
                    stop=not has_bproj,
                    perf_mode=DR,
                )
                if has_bproj:
                    nc.tensor.matmul(
                        p_ps[:], ones_row[:], bproj[:], start=False, stop=True
                    )
                # h1 = x + proj_out  (f32, token-major)
                nc.vector.scalar_tensor_tensor(
                    out=h1_sb[:, i, :],
                    in0=p_ps[:],
                    scalar=INV_PROJ,
                    in1=x_sb[:, i, :],
                    op0=ALU.mult,
                    op1=ALU.add,
                )
                st = stats.tile([P, 6], F32, tag="bn", name=f"st{i}")
                nc.vector.bn_stats(out=st[:], in_=h1_sb[:, i, :])
                nc.vector.bn_aggr(out=mv32[:, i, :], in_=st[:])

            def ln2_group(g):
                for s in range(4):
                    i = 4 * g + s
                    x2 = work.tile([P, C], BF16, tag="x2")
                    ln_normalize(
                        h1_sb[:, i, :],
                        x2[:],
                        mv32[:, i, :],
                        rs32[:, i : i + 1],
                        nc.vector if s % 2 == 0 else nc.gpsimd,
                    )
                    pe_transpose_to(x2T8[:, :, i * P : (i + 1) * P], x2[:], s)

            def fc1_half(hf):
                # hidden rows, fp8 DoubleRow; 2 token-quarters per half
                for hc in range(8):
                    for tq in range(2):
                        q0 = (2 * hf + tq) * 1024
                        f_ps = ps_f.tile([P, 1024], F32, tag="f")
                        for u in range(2):
                            nc.tensor.matmul(
                                f_ps[:, u * 512 : (u + 1) * 512],
                                w18[:, :, hc * P : (hc + 1) * P],
                                x2T8[:, :, q0 + u * 512 : q0 + (u + 1) * 512],
                                start=True,
                                stop=True,
                                perf_mode=DR,
                            )
                        nc.scalar.activation(
                            out=g1T8[:, hc, q0 : q0 + 1024],
                            in_=f_ps[:],
                            func=AF.Gelu,
                            bias=b1[:, hc : hc + 1],
                            scale=1.0 / (S_X2 * s_w1),
                        )

            def fc2_half(hf):
                # feature-major: stationary = w2 pair slices (8 ldweights)
                for cs in range(2):
                    mps = [
                        ps_m.tile([P, 512], F32, tag=f"m{j}", name=f"mp{hf}{cs}{j}")
                        for j in range(4)
                    ]
                    for j in range(4):
                        for tch in range(4):
                            t0 = (4 * hf + tch) * 512
                            nc.tensor.matmul(
                                mps[tch][:],
                                w28[:, 2 * j : 2 * j + 2, cs * P : (cs + 1) * P],
                                g1T8[:, 2 * j : 2 * j + 2, t0 : t0 + 512],
                                start=(j == 0),
                                stop=(j == 3),
                                perf_mode=DR,
                            )
                    for tch in range(4):
                        t0 = (4 * hf + tch) * 512
                        nc.vector.tensor_scalar(
                            out=mTb[:, cs, t0 : t0 + 512],
                            in0=mps[tch][:],
                            scalar1=1.0 / s_w2,
                            scalar2=bfc2[:, cs : cs + 1],
                            op0=ALU.mult,
                            op1=ALU.add,
                        )

            def out_half(hf):
                for g in range(4 * hf, 4 * hf + 4):
                    och = outp.tile([P, 4, C], F32, tag="oc")
                    for s in range(4):
                        i = 4 * g + s
                        tp = ps_t.tile([P, C], BF16, tag="tp")
                        for c in range(2):
                            nc.tensor.transpose(
                                tp[:, c * P : (c + 1) * P],
                                mTb[:, c, i * P : (i + 1) * P],
                                ident[:],
                            )
                        t1 = outp.tile([P, C], F32, tag="t1")
                        nc.vector.tensor_tensor(
                            out=t1[:], in0=tp[:], in1=h1_sb[:, i, :], op=ALU.add
                        )
                        nc.gpsimd.tensor_tensor(
                            out=och[:, s, :], in0=t1[:], in1=x_sb[:, i, :],
                            op=ALU.add,
                        )
                    nc.sync.dma_start(
                        out=out_d[512 * g : 512 * (g + 1), :].rearrange(
                            "(s p) c -> p s c", p=P
                        ),
                        in_=och[:],
                    )

            def half_rstd(hf):
                sl = slice(16 * hf, 16 * (hf + 1))
                ln_rstd(mv32[:, sl, 1], rs32[:, sl], 16, lnsx2_t[:], f"c{hf}")

            # half 0
            for i in range(16):
                c1_tile(i)
            half_rstd(0)
            for g in range(4):
                ln2_group(g)
            fc1_half(0)
            # half 1 (overlaps gelu window of half 0)
            for i in range(16, 32):
                c1_tile(i)
            half_rstd(1)
            for g in range(4, 8):
                ln2_group(g)
            fc2_half(0)
            out_half(0)
            fc1_half(1)
            fc2_half(1)
            out_half(1)

            if DBG:
                for k, src in {
                    "xhT": xhT,
                    "qT": qT8,
                    "E": E8,
                    "h1": h1_sb,
                    "x2T": x2T8,
                    "g1T": g1T8,
                    "mT": mTb,
                }.items():
                    nc.sync.dma_start(
                        out=dbg_d[k][:], in_=src[:].rearrange("p a b -> p (a b)")
                    )

    _split_sync_waits(nc)
    return nc


_CACHE = {}


def _get_nc(key):
    if key not in _CACHE:
        _CACHE[key] = _build_nc(*key)
    return _CACHE[key]


def _pow2_floor(x):
    return float(2.0 ** np.floor(np.log2(x)))


def _prep_inputs(inputs):
    f32 = lambda k: np.asarray(inputs[k], dtype=np.float32)
    qkv_w, qkv_b = f32("qkv_w"), f32("qkv_b")
    proj_w, proj_b = f32("proj_w"), f32("proj_b")
    ln1_g, ln1_b = f32("ln1_g"), f32("ln1_b")
    ln2_g, ln2_b = f32("ln2_g"), f32("ln2_b")
    fc1_w, fc1_b = f32("fc1_w"), f32("fc1_b")
    fc2_w, fc2_b = f32("fc2_w"), f32("fc2_b")

    scale = HD ** (-0.5)

    # Fold LN1 affine into qkv: LN1(x)@W+b = xhat@(g*W) + (ln1_b@W + b)
    wqkv_f = ln1_g[:, None] * qkv_w
    bqkv_f = ln1_b @ qkv_w + qkv_b
    # Fold channel-attention scale into k, then fp8 range scales into k/v/q
    wk = wqkv_f[:, C : 2 * C] * (scale * S_K)
    wv = wqkv_f[:, 2 * C : 3 * C] * S_V
    wqs = wqkv_f[:, 0:C] * S_Q
    bk = bqkv_f[C : 2 * C] * (scale * S_K)
    bv = bqkv_f[2 * C : 3 * C] * S_V
    bqs = bqkv_f[0:C] * S_Q
    # Fold LN2 affine into fc1
    w1_f = ln2_g[:, None] * fc1_w
    b1_f = ln2_b @ fc1_w + fc1_b

    s_w1 = _pow2_floor(FP8_SAFE / max(np.abs(w1_f).max(), 1e-30))
    s_w2 = _pow2_floor(FP8_SAFE / max(np.abs(fc2_w).max(), 1e-30))

    wkv = np.concatenate([wk, wv], axis=1)  # [256, 512]
    bkv = np.concatenate([bk, bv])

    has_bkv = bool(np.any(bkv != 0))
    has_bproj = bool(np.any(proj_b != 0))

    shared = {
        "wkv": wkv.reshape(2, P, 2 * C).astype(NP_BF16),
        "wq": wqs.reshape(2, P, C).astype(NP_BF16),
        "wproj": proj_w.reshape(2, P, C).astype(NP_BF16),
        "w1": (w1_f * s_w1).reshape(2, P, HID).astype(NP_FP8),
        "w2": (fc2_w * s_w2).reshape(8, P, C).astype(NP_FP8),
        "bq": bqs.reshape(2, P, 1).astype(np.float32),
        "b1": b1_f.reshape(8, P, 1).astype(np.float32),
        "bfc2": fc2_b.reshape(2, P, 1).astype(np.float32),
        "bkv": bkv.reshape(1, 2 * C).astype(NP_BF16),
        "bproj": proj_b.reshape(1, C).astype(NP_BF16),
    }
    return shared, (has_bkv, has_bproj, s_w1, s_w2)


def kernel(x, **weights):
    x = np.asarray(x, dtype=np.float32)
    shared, key = _prep_inputs(weights)
    nc = _get_nc(key)
    in_maps = [dict(shared, x=np.ascontiguousarray(x[b])) for b in range(B)]
    res = run_bass_kernel_spmd(nc, in_maps, list(range(B)))
    out = np.stack([res.results[b]["out"] for b in range(B)], axis=0)
    return out.astype(np.float32)


# revision 24
# speedup vs baseline: 1.8105x; 1.2154x over previous
"""ChannelBlock (dense transformer block with channel/cross-covariance attention)
Trainium2 Bass kernel, data-parallel over batch across 8 NeuronCores.

Contract: kernel(**inputs) takes FULL unsharded inputs (np arrays), returns the
FULL output [8, 4096, 256] float32.

v2.2 design notes (per-core, one batch element):
 - channel attention via the Gram matrix: k^T v = Wk^T (xh^T xh) Wv, so only
   G = xh^T xh is accumulated over tokens (PE) and no k/v tensors are ever
   materialized or evicted.
 - proj, fc1 and fc2 run fp8e4 DoubleRow; everything else bf16.
 - activation transposes use PE transpose-mode (bf16 in -> bf16 psum),
   batched 4 tiles per psum tile so evictions are few and large.
 - fc2 is computed feature-major (stationary = w2 pair slices) and
   transposed back via the PE.
 - LN rstd = exp(-0.5*ln(var+eps)): ln+exp share one ACT table set; the only
   ACT table switch in the kernel is to gelu for the MLP.
 - LN normalize alternates DVE / ACT (ACT Identity with per-partition
   scale=rstd, bias=-mean*rstd).
"""

import os

import numpy as np

import concourse.bass as bass
import concourse.bass_utils as _bu
import concourse.tile as tile
from concourse import masks, mybir
from concourse.bass_utils import run_bass_kernel_spmd
from concourse.vector_clock import ScopedClock
import bass_rust

# Optionally re-enable walrus' LDWEIGHTS optimization (off by default in this
# container's compile driver); gated so it can be A/B tested.
if os.environ.get("BASS_LDW_OPT", "0") == "1" and not getattr(
    _bu, "_ldw_patched", False
):
    _orig_run_command = _bu.run_command

    def _run_command_ldw(cmd, **kw):
        if isinstance(cmd, list):
            cmd = [
                "--enable-ldw-opt=true" if c == "--enable-ldw-opt=false" else c
                for c in cmd
            ]
        return _orig_run_command(cmd, **kw)

    _bu.run_command = _run_command_ldw
    _bu._ldw_patched = True

# ----------------------------------------------------------------------------
# Workaround: this container's walrus (CoreV3) only supports ONE sync-wait
# command on TPB_CTRL instructions (Drain).  Tile's kernel-tail drain piles all
# outstanding proc waits onto a single Drain -> split into a chain of Drains
# with one wait each.
# ----------------------------------------------------------------------------
_MAX_DRAIN_WAITS = 1


def _patched_drain_and_barrier(self, tick_clock, wait_clock):
    drain_inst = self.nc.sync.drain()
    wait_clock.add_sem_waits(
        drain_inst.ins, ScopedClock({None: tick_clock.global_clock})
    )
    mi = drain_inst.ins
    si = mi.sync_info
    waits = list(si.on_wait) if si else []
    if len(waits) > _MAX_DRAIN_WAITS:
        mi.sync_info = bass_rust.SyncInfo(
            on_wait=waits[:_MAX_DRAIN_WAITS], on_update=list(si.on_update)
        )
        for i in range(_MAX_DRAIN_WAITS, len(waits), _MAX_DRAIN_WAITS):
            extra = self.nc.sync.drain()
            extra.ins.sync_info = bass_rust.SyncInfo(
                on_wait=waits[i : i + _MAX_DRAIN_WAITS], on_update=[]
            )
    self.nc.all_engine_barrier()
    popped = self.nc._tile_sem_poison_stack.pop()
    assert popped is self._sem_poison
    self.nc.clear_and_free_semaphores(list(self.sems.allocated().values()))
    self.nc.all_engine_barrier()


tile.TileContext._drain_and_barrier = _patched_drain_and_barrier

_nop_counter = [0]


def _split_sync_waits(nc, cap=1):
    """Walrus in this container rejects instructions with more than `cap`
    sync-wait commands.  Hoist excess waits onto same-engine NOPs inserted
    immediately before the instruction (engine streams are in-order, so the
    semantics are unchanged)."""
    for f in nc.m.functions:
        for blk in f.blocks:
            changed = False
            new = []
            for inst in blk.instructions:
                si = inst.sync_info
                waits = list(si.on_wait) if si is not None else []
                # ldw-opt rejects Ldweights carrying sync waits; hoist them.
                is_ldw = inst.__class__.__name__ == "InstLdweights"
                eff_cap = 0 if (is_ldw and waits) else cap
                if len(waits) > eff_cap:
                    if is_ldw:
                        excess, keep = waits, []
                    else:
                        excess, keep = waits[:-cap], waits[-cap:]
                    for j in range(0, len(excess), cap):
                        _nop_counter[0] += 1
                        nop = mybir.InstNoOp(
                            name=f"NW-{_nop_counter[0]}", ins=[], outs=[]
                        )
                        nop.engine = inst.engine
                        nop.sync_info = bass_rust.SyncInfo(
                            on_wait=excess[j : j + cap], on_update=[]
                        )
                        new.append(nop)
                    inst.sync_info = bass_rust.SyncInfo(
                        on_wait=keep, on_update=list(si.on_update)
                    )
                    changed = True
                new.append(inst)
            if changed:
                blk.instructions = new


# ----------------------------------------------------------------------------
# Problem constants (hardcoded per the task contract)
# ----------------------------------------------------------------------------
B = 8
N = 4096
C = 256
H = 8
HD = C // H  # 32
HID = 1024
EPS = 1e-5
P = 128
NTILES = N // P  # 32
NG = NTILES // 4  # 8 groups of 4 tiles (512 tokens each)

F32 = mybir.dt.float32
BF16 = mybir.dt.bfloat16
FP8 = mybir.dt.float8e4
NP_BF16 = mybir.dt.np(BF16)
NP_FP8 = mybir.dt.np(FP8)
FP8_SAFE = 224.0  # ml_dtypes float8_e4m3 max finite is 240

AF = mybir.ActivationFunctionType
ALU = mybir.AluOpType
AX = mybir.AxisListType
DR = mybir.MatmulPerfMode.DoubleRow

# activation scales (power-of-two; fp8 range management only)
S_Q = 16.0
S_X2 = 16.0
S_E = 256.0
INV_PROJ = 1.0 / (S_Q * S_E)
LN_SX2 = float(np.log(S_X2))


def _build_nc(has_bkv, has_bproj, s_w1, s_w2):
    nc = bass.Bass()

    # ---- DRAM I/O ----
    x_d = nc.declare_dram_parameter("x", [N, C], F32, isOutput=False)
    wkv_d = nc.declare_dram_parameter("wkv", [2, P, 2 * C], BF16, isOutput=False)
    wq_d = nc.declare_dram_parameter("wq", [2, P, C], BF16, isOutput=False)
    wproj_d = nc.declare_dram_parameter("wproj", [2, P, C], BF16, isOutput=False)
    w1_d = nc.declare_dram_parameter("w1", [2, P, HID], FP8, isOutput=False)
    w2_d = nc.declare_dram_parameter("w2", [8, P, C], FP8, isOutput=False)
    bq_d = nc.declare_dram_parameter("bq", [2, P, 1], F32, isOutput=False)
    b1_d = nc.declare_dram_parameter("b1", [8, P, 1], F32, isOutput=False)
    bfc2_d = nc.declare_dram_parameter("bfc2", [2, P, 1], F32, isOutput=False)
    bkv_d = nc.declare_dram_parameter("bkv", [2, 2 * C], BF16, isOutput=False)
    bproj_d = nc.declare_dram_parameter("bproj", [1, C], BF16, isOutput=False)
    out_d = nc.declare_dram_parameter("out", [N, C], F32, isOutput=True)
    DBG = os.environ.get("BASS_DBG", "0") == "1"
    if DBG:
        dbg_d = {
            k: nc.declare_dram_parameter(f"dbg_{k}", shp, dt, isOutput=True)
            for k, (shp, dt) in {
                "xhT": ([P, 2 * N], BF16),
                "qT": ([P, 2 * N], FP8),
                "E": ([P, 2 * C], FP8),
                "h1": ([P, NTILES * C], F32),
                "x2T": ([P, 2 * N], FP8),
                "g1T": ([P, 8 * N], FP8),
                "mT": ([P, 2 * N], BF16),
            }.items()
        }

    with tile.TileContext(nc) as tc:
        import contextlib

        ctx = contextlib.ExitStack()
        with ctx:
            const = ctx.enter_context(tc.tile_pool(name="const", bufs=1))
            xres = ctx.enter_context(tc.tile_pool(name="xres", bufs=1))
            stats = ctx.enter_context(tc.tile_pool(name="stats", bufs=4))
            work = ctx.enter_context(tc.tile_pool(name="work", bufs=6))
            outp = ctx.enter_context(tc.tile_pool(name="outp", bufs=3))
            # transpose (bf16, 4 tiles batched) + c1 psum pool; lives all kernel
            ps_t = ctx.enter_context(tc.tile_pool(name="ps_t", bufs=1, space="PSUM"))

            # ---- residents ----
            x_sb = xres.tile([P, NTILES, C], F32)  # raw x, token-major
            h1_sb = xres.tile([P, NTILES, C], F32)  # x + attn, token-major
            xhT = xres.tile([P, 2, N], BF16)  # LN1(x)^T  (feature-major)
            qT8 = xres.tile([P, 2, N], FP8)  # (q*S_Q)^T
            x2T8 = xres.tile([P, 2, N], FP8)  # (LN2(h1)*S_X2)^T
            g1T8 = xres.tile([P, 8, N], FP8)  # gelu(fc1)^T
            mTb = xres.tile([P, 2, N], BF16)  # fc2 out, feature-major
            mv32 = xres.tile([P, NTILES, 2], F32)
            rs32 = xres.tile([P, NTILES], F32)
            nmr32 = xres.tile([P, NTILES], F32)  # -mean*rstd for LN2

            # ---- input DMAs: x first (compute starts on it), weights after --
            for g in range(NG):
                nc.sync.dma_start(
                    out=x_sb[:, 4 * g : 4 * g + 4, :],
                    in_=x_d[512 * g : 512 * (g + 1), :].rearrange(
                        "(s p) c -> p s c", p=P
                    ),
                )
            wkv = const.tile([P, 2, 2 * C], BF16)
            wq = const.tile([P, 2, C], BF16)
            wproj = const.tile([P, 2, C], BF16)
            bq = const.tile([P, 2], F32)
            nc.sync.dma_start(out=wkv[:], in_=wkv_d.rearrange("c p f -> p c f"))
            nc.sync.dma_start(out=wq[:], in_=wq_d.rearrange("c p f -> p c f"))
            nc.sync.dma_start(out=wproj[:], in_=wproj_d.rearrange("c p f -> p c f"))
            for c in range(2):
                nc.sync.dma_start(out=bq[:, c : c + 1], in_=bq_d[c])
            w18 = const.tile([P, 2, HID], FP8)
            w28 = const.tile([P, 8, C], FP8)
            b1 = const.tile([P, 8], F32)
            bfc2 = const.tile([P, 2], F32)
            nc.sync.dma_start(out=w18[:], in_=w1_d.rearrange("c p f -> p c f"))
            nc.sync.dma_start(out=w28[:], in_=w2_d.rearrange("c p f -> p c f"))
            for c in range(8):
                nc.sync.dma_start(out=b1[:, c : c + 1], in_=b1_d[c])
            for c in range(2):
                nc.sync.dma_start(out=bfc2[:, c : c + 1], in_=bfc2_d[c])
            ident = const.tile([P, P], BF16)
            masks.make_identity(nc, ident[:])
            ones_row = const.tile([1, P], BF16)
            nc.vector.memset(ones_row[:], 1.0)
            ones_col = const.tile([P, 1], BF16)
            nc.vector.memset(ones_col[:], 1.0)
            eps_t = const.tile([P, 1], F32)
            nc.vector.memset(eps_t[:], EPS)
            lnsx2_t = const.tile([P, 1], F32)
            nc.vector.memset(lnsx2_t[:], LN_SX2)
            bkv = const.tile([2, 2 * C], BF16)  # [bk|bv] ; row1 = bk_sv_nv row
            bproj = const.tile([1, C], BF16)
            if has_bkv:
                nc.sync.dma_start(out=bkv[:], in_=bkv_d[:])
            if has_bproj:
                nc.sync.dma_start(out=bproj[:], in_=bproj_d[:])

            # ---- helpers ----
            def ln_rstd(var_ap, mu_ap, rs_ap, nmr_ap, nsub, logmul_ap, tag):
                # rs = exp(-0.5*ln(var+eps) + logmul) == exp(logmul)/sqrt(var+eps)
                lnv = stats.tile([P, nsub], F32, tag=f"lnv{tag}")
                nc.scalar.activation(
                    out=lnv[:], in_=var_ap, func=AF.Ln, bias=eps_t[:]
                )
                if logmul_ap is None:
                    nc.scalar.activation(
                        out=rs_ap, in_=lnv[:], func=AF.Exp, scale=-0.5
                    )
                else:
                    nc.scalar.activation(
                        out=rs_ap, in_=lnv[:], func=AF.Exp, scale=-0.5,
                        bias=logmul_ap,
                    )
                # nmr = -mean * rs
                nc.vector.scalar_tensor_tensor(
                    out=nmr_ap, in0=mu_ap, scalar=-1.0, in1=rs_ap,
                    op0=ALU.mult, op1=ALU.mult,
                )

            def ln_normalize(src_ap, dst, rs1, nmr1, s):
                # dst = src*rs + (-mu*rs); alternate DVE / ACT
                if s % 2 == 0:
                    nc.vector.tensor_scalar(
                        out=dst, in0=src_ap, scalar1=rs1, scalar2=nmr1,
                        op0=ALU.mult, op1=ALU.add,
                    )
                else:
                    nc.scalar.activation(
                        out=dst, in_=src_ap, func=AF.Identity, scale=rs1,
                        bias=nmr1,
                    )

            # =============== Phase A: LN1, xhat^T, Gram accum, q^T ===========
            ab_ctx = contextlib.ExitStack()
            ps_G = ab_ctx.enter_context(
                tc.tile_pool(name="ps_G", bufs=1, space="PSUM")
            )
            G_ps = [ps_G.tile([P, C], F32, name=f"G{i}") for i in range(2)]
            s_ps = (
                [ps_G.tile([P, 1], F32, name=f"s{i}") for i in range(2)]
                if has_bkv
                else None
            )
            with tc.tile_pool(name="ps_q", bufs=2, space="PSUM") as ps_q:

                def q_pass(p):
                    # q^T for token chunks 4p..4p+3 (feature-major, fp8 out)
                    for fc in range(2):
                        for j in range(4):
                            ch = 4 * p + j
                            qps = ps_q.tile([P, 512], F32, tag="q")
                            for kc in range(2):
                                nc.tensor.matmul(
                                    qps[:],
                                    wq[:, kc, fc * P : (fc + 1) * P],
                                    xhT[:, kc, ch * 512 : (ch + 1) * 512],
                                    start=(kc == 0),
                                    stop=(kc == 1),
                                )
                            # qT8 = psum + bq_scaled   (cast fp8)
                            nc.scalar.activation(
                                out=qT8[:, fc, ch * 512 : (ch + 1) * 512],
                                in_=qps[:],
                                func=AF.Identity,
                                bias=bq[:, fc : fc + 1],
                            )

                for g in range(NG):
                    idxs = [4 * g + s for s in range(4)]
                    mv4 = stats.tile([P, 4, 2], F32, tag="mv")
                    rs4 = stats.tile([P, 4], F32, tag="rs")
                    nmr4 = stats.tile([P, 4], F32, tag="nmr")
                    for s, i in enumerate(idxs):
                        st = stats.tile([P, 6], F32, tag="bn")
                        nc.vector.bn_stats(out=st[:], in_=x_sb[:, i, :])
                        nc.vector.bn_aggr(out=mv4[:, s, :], in_=st[:])
                    ln_rstd(
                        mv4[:, :, 1], mv4[:, :, 0], rs4[:], nmr4[:], 4, None, "a"
                    )
                    tp4 = ps_t.tile([P, 4, C], BF16, tag="tp")
                    xh4 = [None] * 4
                    for s, i in enumerate(idxs):
                        xhat = work.tile([P, C], BF16, tag="xhat")
                        xh4[s] = xhat
                        ln_normalize(
                            x_sb[:, i, :], xhat[:], rs4[:, s : s + 1],
                            nmr4[:, s : s + 1], s,
                        )
                        for c in range(2):
                            nc.tensor.transpose(
                                tp4[:, s, c * P : (c + 1) * P],
                                xhat[:, c * P : (c + 1) * P],
                                ident[:],
                            )
                        # Gram accumulation: G[c] += xhat[:,c-half].T @ xhat
                        tile_i = 4 * g + s
                        for c in range(2):
                            nc.tensor.matmul(
                                G_ps[c][:],
                                xhat[:, c * P : (c + 1) * P],
                                xhat[:, :],
                                start=(tile_i == 0),
                                stop=(tile_i == NTILES - 1),
                            )
                        if has_bkv:
                            for c in range(2):
                                nc.tensor.matmul(
                                    s_ps[c][:],
                                    xhat[:, c * P : (c + 1) * P],
                                    ones_col[:],
                                    start=(tile_i == 0),
                                    stop=(tile_i == NTILES - 1),
                                )
                    # batched eviction of 4 transposed tiles -> xhT
                    nc.vector.tensor_copy(
                        out=xhT[:, :, g * 512 : (g + 1) * 512].rearrange(
                            "p c (s t) -> p c s t", s=4
                        ),
                        in_=tp4[:].rearrange("p s (c t) -> p c s t", c=2),
                    )
                    if g == 3:
                        q_pass(0)
                    elif g == 7:
                        q_pass(1)

            # =============== Phase B: logits = Wk^T G Wv, softmax -> E =======
            BdT = const.tile([P, 2, P], BF16)
            nc.vector.memset(BdT[:], 0.0)
            E8 = const.tile([P, 2, C], FP8)
            with tc.tile_pool(name="ps_b", bufs=2, space="PSUM") as ps_b, \
                 tc.tile_pool(name="ps_L", bufs=1, space="PSUM") as ps_L:
                g_sb = work.tile([P, 2, C], BF16, tag="gsb")
                for c in range(2):
                    nc.vector.tensor_copy(out=g_sb[:, c, :], in_=G_ps[c][:])
                # U = G @ Wv  (uses G symmetry: lhsT slice of g_sb)
                u_sb = work.tile([P, 2, C], BF16, tag="usb")
                for fo in range(2):
                    u_ps = ps_b.tile([P, C], F32, tag="b")
                    for cp in range(2):
                        nc.tensor.matmul(
                            u_ps[:],
                            g_sb[:, cp, fo * P : (fo + 1) * P],
                            wkv[:, cp, C : 2 * C],
                            start=(cp == 0),
                            stop=(cp == 1),
                        )
                    nc.vector.tensor_copy(out=u_sb[:, fo, :], in_=u_ps[:])
                # L = Wk^T U  -> one psum bank, halves at col 0/256
                L_tile = ps_L.tile([P, 2 * C], F32, name="L")
                L_ps = [L_tile[:, i * C : (i + 1) * C] for i in range(2)]
                for ko in range(2):
                    for cp in range(2):
                        nc.tensor.matmul(
                            L_ps[ko],
                            wkv[:, cp, ko * P : (ko + 1) * P],
                            u_sb[:, cp, :],
                            start=(cp == 0),
                            stop=(cp == 1),
                        )
                if has_bkv:
                    # bias corrections: L += bk (s^T Wv + N bv) + (Wk^T s) bv^T
                    s_sb = work.tile([P, 2], BF16, tag="ssb")
                    for c in range(2):
                        nc.vector.tensor_copy(out=s_sb[:, c : c + 1], in_=s_ps[c][:])
                    svr = work.tile([1, C], F32, tag="svr")
                    sv_ps = ps_b.tile([1, C], F32, tag="sv")
                    for cp in range(2):
                        nc.tensor.matmul(
                            sv_ps[:],
                            s_sb[:, cp : cp + 1],
                            wkv[:, cp, C : 2 * C],
                            start=(cp == 0),
                            stop=(cp == 1),
                        )
                    # svr = s^T Wv + N*bv   (bkv row1 holds N*bv host-side)
                    nc.vector.tensor_tensor(
                        out=svr[:], in0=sv_ps[:], in1=bkv[1:2, C : 2 * C],
                        op=ALU.add,
                    )
                    svr_b = work.tile([1, C], BF16, tag="svrb")
                    nc.vector.tensor_copy(out=svr_b[:], in_=svr[:])
                    sk_ps = ps_b.tile([1, C], F32, tag="sk")
                    for cp in range(2):
                        nc.tensor.matmul(
                            sk_ps[:],
                            s_sb[:, cp : cp + 1],
                            wkv[:, cp, 0:C],
                            start=(cp == 0),
                            stop=(cp == 1),
                        )
                    skr = work.tile([1, C], BF16, tag="skr")
                    nc.vector.tensor_copy(out=skr[:], in_=sk_ps[:])
                    for ko in range(2):
                        nc.tensor.matmul(
                            L_ps[ko],
                            bkv[0:1, C + ko * P : C + (ko + 1) * P],
                            svr_b[:],
                            start=False,
                            stop=False,
                        )
                        nc.tensor.matmul(
                            L_ps[ko],
                            skr[0:1, ko * P : (ko + 1) * P],
                            bkv[0:1, C : 2 * C],
                            start=False,
                            stop=True,
                        )
                for half in range(2):
                    a_sb = work.tile([P, HD], F32, tag="attn")
                    for h in range(4):
                        hh = half * 4 + h
                        nc.vector.tensor_copy(
                            out=a_sb[h * HD : (h + 1) * HD, :],
                            in_=L_tile[
                                h * HD : (h + 1) * HD,
                                half * C + hh * HD : half * C + (hh + 1) * HD,
                            ],
                        )
                    negmax = stats.tile([P, 1], F32, tag="negmax")
                    nc.vector.tensor_reduce(
                        out=negmax[:], in_=a_sb[:], axis=AX.X, op=ALU.max,
                        negate=True,
                    )
                    exps = work.tile([P, HD], F32, tag="exps")
                    nc.scalar.activation(
                        out=exps[:], in_=a_sb[:], func=AF.Exp, bias=negmax[:]
                    )
                    ssum = stats.tile([P, 1], F32, tag="ssum")
                    nc.vector.tensor_reduce(
                        out=ssum[:], in_=exps[:], axis=AX.X, op=ALU.add
                    )
                    rec = stats.tile([P, 1], F32, tag="rec")
                    nc.vector.reciprocal(out=rec[:], in_=ssum[:])
                    for h in range(4):
                        sl = slice(h * HD, (h + 1) * HD)
                        nc.vector.tensor_scalar(
                            out=BdT[sl, half, sl],
                            in0=exps[sl, :],
                            scalar1=rec[sl, 0:1],
                            scalar2=None,
                            op0=ALU.mult,
                        )
                for half in range(2):
                    e_ps = ps_b.tile([P, C], F32, tag="b")
                    nc.tensor.matmul(
                        e_ps[:],
                        BdT[:, half, :],
                        wproj[:, half, :],
                        start=True,
                        stop=True,
                    )
                    nc.vector.tensor_scalar(
                        out=E8[:, half, :],
                        in0=e_ps[:],
                        scalar1=S_E,
                        scalar2=None,
                        op0=ALU.mult,
                    )
            ab_ctx.close()  # free Gram psum banks before phase C pools open

            # =============== Phase C: proj+res+LN2 / fc1+gelu / fc2 ==========
            ps_f = ctx.enter_context(tc.tile_pool(name="ps_f", bufs=1, space="PSUM"))
            ps_m = ctx.enter_context(tc.tile_pool(name="ps_m", bufs=1, space="PSUM"))

            def c1_pair(i):
                # proj + residual for tiles i, i+1 in one [P,512] psum tile
                p_ps = ps_t.tile([P, 512], F32, tag="c1", name=f"pp{i}")
                for u in range(2):
                    nc.tensor.matmul(
                        p_ps[:, u * C : (u + 1) * C],
                        qT8[:, :, (i + u) * P : (i + u + 1) * P],
                        E8[:, :, :],
                        start=True,
                        stop=not has_bproj,
                        perf_mode=DR,
                    )
                    if has_bproj:
                        nc.tensor.matmul(
                            p_ps[:, u * C : (u + 1) * C],
                            ones_row[:],
                            bproj[:],
                            start=False,
                            stop=True,
                        )
                # h1 = x + proj_out  (f32, token-major), 2 tiles at once
                nc.vector.scalar_tensor_tensor(
                    out=h1_sb[:, i : i + 2, :],
                    in0=p_ps[:].rearrange("p (u c) -> p u c", u=2),
                    scalar=INV_PROJ,
                    in1=x_sb[:, i : i + 2, :],
                    op0=ALU.mult,
                    op1=ALU.add,
                )
                for u in range(2):
                    st = stats.tile([P, 6], F32, tag="bn", name=f"st{i + u}")
                    nc.vector.bn_stats(out=st[:], in_=h1_sb[:, i + u, :])
                    nc.vector.bn_aggr(out=mv32[:, i + u, :], in_=st[:])

            def ln2_group(g):
                tp4 = ps_t.tile([P, 4, C], BF16, tag="tp")
                for s in range(4):
                    i = 4 * g + s
                    x2 = work.tile([P, C], BF16, tag="x2")
                    ln_normalize(
                        h1_sb[:, i, :], x2[:], rs32[:, i : i + 1],
                        nmr32[:, i : i + 1], s,
                    )
                    for c in range(2):
                        nc.tensor.transpose(
                            tp4[:, s, c * P : (c + 1) * P],
                            x2[:, c * P : (c + 1) * P],
                            ident[:],
                        )
                # batched eviction -> x2T8 (fp8)
                nc.scalar.copy(
                    out=x2T8[:, :, g * 512 : (g + 1) * 512].rearrange(
                        "p c (s t) -> p c s t", s=4
                    ),
                    in_=tp4[:].rearrange("p s (c t) -> p c s t", c=2),
                )

            def fc1_half(hf):
                # hidden rows, fp8 DoubleRow; 2 token-quarters per half
                for hc in range(8):
                    for tq in range(2):
                        q0 = (2 * hf + tq) * 1024
                        f_ps = ps_f.tile([P, 1024], F32, tag="f")
                        for u in range(2):
                            nc.tensor.matmul(
                                f_ps[:, u * 512 : (u + 1) * 512],
                                w18[:, :, hc * P : (hc + 1) * P],
                                x2T8[:, :, q0 + u * 512 : q0 + (u + 1) * 512],
                                start=True,
                                stop=True,
                                perf_mode=DR,
                            )
                        nc.scalar.activation(
                            out=g1T8[:, hc, q0 : q0 + 1024],
                            in_=f_ps[:],
                            func=AF.Gelu,
                            bias=b1[:, hc : hc + 1],
                            scale=1.0 / (S_X2 * s_w1),
                        )

            def fc2_half(hf):
                # feature-major: stationary = w2 pair slices
                for cs in range(2):
                    for ph in range(2):
                        mps = [
                            ps_m.tile(
                                [P, 512], F32, tag=f"m{j}",
                                name=f"mp{hf}{cs}{ph}{j}",
                            )
                            for j in range(2)
                        ]
                        for j in range(4):
                            for u in range(2):
                                tch = 2 * ph + u
                                t0 = (4 * hf + tch) * 512
                                nc.tensor.matmul(
                                    mps[u][:],
                                    w28[:, 2 * j : 2 * j + 2, cs * P : (cs + 1) * P],
                                    g1T8[:, 2 * j : 2 * j + 2, t0 : t0 + 512],
                                    start=(j == 0),
                                    stop=(j == 3),
                                    perf_mode=DR,
                                )
                        for u in range(2):
                            tch = 2 * ph + u
                            t0 = (4 * hf + tch) * 512
                            nc.scalar.activation(
                                out=mTb[:, cs, t0 : t0 + 512],
                                in_=mps[u][:],
                                func=AF.Identity,
                                scale=1.0 / s_w2,
                                bias=bfc2[:, cs : cs + 1],
                            )

            def out_half(hf):
                for g in range(4 * hf, 4 * hf + 4):
                    och = outp.tile([P, 4, C], F32, tag="oc")
                    for pr in range(2):
                        i = 4 * g + 2 * pr
                        tp = ps_t.tile([P, 2, C], BF16, tag="tpo")
                        for u in range(2):
                            for c in range(2):
                                nc.tensor.transpose(
                                    tp[:, u, c * P : (c + 1) * P],
                                    mTb[:, c, (i + u) * P : (i + u + 1) * P],
                                    ident[:],
                                )
                        t1 = outp.tile([P, 2, C], F32, tag="t1")
                        nc.vector.tensor_tensor(
                            out=t1[:], in0=tp[:], in1=h1_sb[:, i : i + 2, :],
                            op=ALU.add,
                        )
                        nc.gpsimd.tensor_tensor(
                            out=och[:, 2 * pr : 2 * pr + 2, :],
                            in0=t1[:],
                            in1=x_sb[:, i : i + 2, :],
                            op=ALU.add,
                        )
                    nc.sync.dma_start(
                        out=out_d[512 * g : 512 * (g + 1), :].rearrange(
                            "(s p) c -> p s c", p=P
                        ),
                        in_=och[:],
                    )

            def half_rstd(hf):
                sl = slice(16 * hf, 16 * (hf + 1))
                ln_rstd(
                    mv32[:, sl, 1], mv32[:, sl, 0], rs32[:, sl], nmr32[:, sl],
                    16, lnsx2_t[:], f"c{hf}",
                )

            # C1 for all tiles first (keeps ln/exp ACT table resident), then
            # the MLP pipeline in two half-N passes: fc1(H0)'s gelu window
            # overlaps LN2(H1); fc1(H1)'s overlaps fc2(H0)+outputs(H0).
            for i in range(0, 16, 2):
                c1_pair(i)
            half_rstd(0)
            for i in range(16, 32, 2):
                c1_pair(i)
            half_rstd(1)
            for g in range(4):
                ln2_group(g)
            fc1_half(0)
            for g in range(4, 8):
                ln2_group(g)
            fc2_half(0)
            out_half(0)
            fc1_half(1)
            fc2_half(1)
            out_half(1)

            if DBG:
                for k, src in {
                    "xhT": xhT,
                    "qT": qT8,
                    "E": E8,
                    "h1": h1_sb,
                    "x2T": x2T8,
                    "g1T": g1T8,
                    "mT": mTb,
                }.items():
                    nc.sync.dma_start(
                        out=dbg_d[k][:], in_=src[:].rearrange("p a b -> p (a b)")
                    )

    _split_sync_waits(nc)
    return nc


_CACHE = {}


def _get_nc(key):
    if key not in _CACHE:
        _CACHE[key] = _build_nc(*key)
    return _CACHE[key]


def _pow2_floor(x):
    return float(2.0 ** np.floor(np.log2(x)))


def _prep_inputs(inputs):
    f32 = lambda k: np.asarray(inputs[k], dtype=np.float32)
    qkv_w, qkv_b = f32("qkv_w"), f32("qkv_b")
    proj_w, proj_b = f32("proj_w"), f32("proj_b")
    ln1_g, ln1_b = f32("ln1_g"), f32("ln1_b")
    ln2_g, ln2_b = f32("ln2_g"), f32("ln2_b")
    fc1_w, fc1_b = f32("fc1_w"), f32("fc1_b")
    fc2_w, fc2_b = f32("fc2_w"), f32("fc2_b")

    scale = HD ** (-0.5)

    # Fold LN1 affine into qkv: LN1(x)@W+b = xhat@(g*W) + (ln1_b@W + b)
    wqkv_f = ln1_g[:, None] * qkv_w
    bqkv_f = ln1_b @ qkv_w + qkv_b
    # Fold channel-attention scale into k; q gets its fp8 range scale
    wk = wqkv_f[:, C : 2 * C] * scale
    wv = wqkv_f[:, 2 * C : 3 * C]
    wqs = wqkv_f[:, 0:C] * S_Q
    bk = bqkv_f[C : 2 * C] * scale
    bv = bqkv_f[2 * C : 3 * C]
    bqs = bqkv_f[0:C] * S_Q
    # Fold LN2 affine into fc1
    w1_f = ln2_g[:, None] * fc1_w
    b1_f = ln2_b @ fc1_w + fc1_b

    s_w1 = _pow2_floor(FP8_SAFE / max(np.abs(w1_f).max(), 1e-30))
    s_w2 = _pow2_floor(FP8_SAFE / max(np.abs(fc2_w).max(), 1e-30))

    wkv = np.concatenate([wk, wv], axis=1)  # [256, 512]
    bkv_row0 = np.concatenate([bk, bv])
    bkv_row1 = np.concatenate([np.zeros(C, np.float32), N * bv])
    bkv = np.stack([bkv_row0, bkv_row1])

    has_bkv = bool(np.any(bkv_row0 != 0))
    has_bproj = bool(np.any(proj_b != 0))

    shared = {
        "wkv": wkv.reshape(2, P, 2 * C).astype(NP_BF16),
        "wq": wqs.reshape(2, P, C).astype(NP_BF16),
        "wproj": proj_w.reshape(2, P, C).astype(NP_BF16),
        "w1": (w1_f * s_w1).reshape(2, P, HID).astype(NP_FP8),
        "w2": (fc2_w * s_w2).reshape(8, P, C).astype(NP_FP8),
        "bq": bqs.reshape(2, P, 1).astype(np.float32),
        "b1": b1_f.reshape(8, P, 1).astype(np.float32),
        "bfc2": fc2_b.reshape(2, P, 1).astype(np.float32),
        "bkv": bkv.astype(NP_BF16),
        "bproj": proj_b.reshape(1, C).astype(NP_BF16),
    }
    return shared, (has_bkv, has_bproj, s_w1, s_w2)


def kernel(x, **weights):
    x = np.asarray(x, dtype=np.float32)
    shared, key = _prep_inputs(weights)
    nc = _get_nc(key)
    in_maps = [dict(shared, x=np.ascontiguousarray(x[b])) for b in range(B)]
    res = run_bass_kernel_spmd(nc, in_maps, list(range(B)))
    out = np.stack([res.results[b]["out"] for b in range(B)], axis=0)
    return out.astype(np.float32)


# revision 25
# speedup vs baseline: 2.5955x; 1.4336x over previous
"""ChannelBlock (dense transformer block with channel/cross-covariance attention)
Trainium2 Bass kernel, data-parallel over batch across 8 NeuronCores.

Contract: kernel(**inputs) takes FULL unsharded inputs (np arrays), returns the
FULL output [8, 4096, 256] float32.

v2.2 design notes (per-core, one batch element):
 - channel attention via the Gram matrix: k^T v = Wk^T (xh^T xh) Wv, so only
   G = xh^T xh is accumulated over tokens (PE) and no k/v tensors are ever
   materialized or evicted.
 - proj, fc1 and fc2 run fp8e4 DoubleRow; everything else bf16.
 - activation transposes use PE transpose-mode (bf16 in -> bf16 psum),
   batched 4 tiles per psum tile so evictions are few and large.
 - fc2 is computed feature-major (stationary = w2 pair slices) and
   transposed back via the PE.
 - LN rstd = exp(-0.5*ln(var+eps)): ln+exp share one ACT table set; the only
   ACT table switch in the kernel is to gelu for the MLP.
 - LN normalize alternates DVE / ACT (ACT Identity with per-partition
   scale=rstd, bias=-mean*rstd).
"""

import os

import numpy as np

import concourse.bass as bass
import concourse.bass_utils as _bu
import concourse.tile as tile
from concourse import masks, mybir
from concourse.bass_utils import run_bass_kernel_spmd
from concourse.vector_clock import ScopedClock
import bass_rust

# Optionally re-enable walrus' LDWEIGHTS optimization (off by default in this
# container's compile driver); gated so it can be A/B tested.
if os.environ.get("BASS_LDW_OPT", "0") == "1" and not getattr(
    _bu, "_ldw_patched", False
):
    _orig_run_command = _bu.run_command

    def _run_command_ldw(cmd, **kw):
        if isinstance(cmd, list):
            cmd = [
                "--enable-ldw-opt=true" if c == "--enable-ldw-opt=false" else c
                for c in cmd
            ]
        return _orig_run_command(cmd, **kw)

    _bu.run_command = _run_command_ldw
    _bu._ldw_patched = True

# ----------------------------------------------------------------------------
# Workaround: this container's walrus (CoreV3) only supports ONE sync-wait
# command on TPB_CTRL instructions (Drain).  Tile's kernel-tail drain piles all
# outstanding proc waits onto a single Drain -> split into a chain of Drains
# with one wait each.
# ----------------------------------------------------------------------------
_MAX_DRAIN_WAITS = 1


def _patched_drain_and_barrier(self, tick_clock, wait_clock):
    drain_inst = self.nc.sync.drain()
    wait_clock.add_sem_waits(
        drain_inst.ins, ScopedClock({None: tick_clock.global_clock})
    )
    mi = drain_inst.ins
    si = mi.sync_info
    waits = list(si.on_wait) if si else []
    if len(waits) > _MAX_DRAIN_WAITS:
        mi.sync_info = bass_rust.SyncInfo(
            on_wait=waits[:_MAX_DRAIN_WAITS], on_update=list(si.on_update)
        )
        for i in range(_MAX_DRAIN_WAITS, len(waits), _MAX_DRAIN_WAITS):
            extra = self.nc.sync.drain()
            extra.ins.sync_info = bass_rust.SyncInfo(
                on_wait=waits[i : i + _MAX_DRAIN_WAITS], on_update=[]
            )
    self.nc.all_engine_barrier()
    popped = self.nc._tile_sem_poison_stack.pop()
    assert popped is self._sem_poison
    self.nc.clear_and_free_semaphores(list(self.sems.allocated().values()))
    self.nc.all_engine_barrier()


tile.TileContext._drain_and_barrier = _patched_drain_and_barrier

_nop_counter = [0]


def _split_sync_waits(nc, cap=1):
    """Walrus in this container rejects instructions with more than `cap`
    sync-wait commands.  Hoist excess waits onto same-engine NOPs inserted
    immediately before the instruction (engine streams are in-order, so the
    semantics are unchanged)."""
    for f in nc.m.functions:
        for blk in f.blocks:
            changed = False
            new = []
            for inst in blk.instructions:
                si = inst.sync_info
                waits = list(si.on_wait) if si is not None else []
                # ldw-opt rejects Ldweights carrying sync waits; hoist them.
                is_ldw = inst.__class__.__name__ == "InstLdweights"
                eff_cap = 0 if (is_ldw and waits) else cap
                if len(waits) > eff_cap:
                    if is_ldw:
                        excess, keep = waits, []
                    else:
                        excess, keep = waits[:-cap], waits[-cap:]
                    for j in range(0, len(excess), cap):
                        _nop_counter[0] += 1
                        nop = mybir.InstNoOp(
                            name=f"NW-{_nop_counter[0]}", ins=[], outs=[]
                        )
                        nop.engine = inst.engine
                        nop.sync_info = bass_rust.SyncInfo(
                            on_wait=excess[j : j + cap], on_update=[]
                        )
                        new.append(nop)
                    inst.sync_info = bass_rust.SyncInfo(
                        on_wait=keep, on_update=list(si.on_update)
                    )
                    changed = True
                new.append(inst)
            if changed:
                blk.instructions = new


# ----------------------------------------------------------------------------
# Problem constants (hardcoded per the task contract)
# ----------------------------------------------------------------------------
B = 8
N = 4096
C = 256
H = 8
HD = C // H  # 32
HID = 1024
EPS = 1e-5
P = 128
NTILES = N // P  # 32
NG = NTILES // 4  # 8 groups of 4 tiles (512 tokens each)

F32 = mybir.dt.float32
BF16 = mybir.dt.bfloat16
FP8 = mybir.dt.float8e4
NP_BF16 = mybir.dt.np(BF16)
NP_FP8 = mybir.dt.np(FP8)
FP8_SAFE = 224.0  # ml_dtypes float8_e4m3 max finite is 240

AF = mybir.ActivationFunctionType
ALU = mybir.AluOpType
AX = mybir.AxisListType
DR = mybir.MatmulPerfMode.DoubleRow

# activation scales (power-of-two; fp8 range management only)
S_Q = 16.0
S_X2 = 16.0
S_E = 256.0
INV_PROJ = 1.0 / (S_Q * S_E)
LN_SX2 = float(np.log(S_X2))


def _build_nc(has_bkv, has_bproj, s_w1, s_w2):
    nc = bass.Bass()

    # ---- DRAM I/O ----
    x_d = nc.declare_dram_parameter("x", [N, C], F32, isOutput=False)
    wkv_d = nc.declare_dram_parameter("wkv", [2, P, 2 * C], BF16, isOutput=False)
    wq_d = nc.declare_dram_parameter("wq", [2, P, C], BF16, isOutput=False)
    wproj_d = nc.declare_dram_parameter("wproj", [2, P, C], BF16, isOutput=False)
    w1_d = nc.declare_dram_parameter("w1", [2, P, HID], FP8, isOutput=False)
    w2_d = nc.declare_dram_parameter("w2", [8, P, C], FP8, isOutput=False)
    bq_d = nc.declare_dram_parameter("bq", [2, P, 1], F32, isOutput=False)
    b1_d = nc.declare_dram_parameter("b1", [8, P, 1], F32, isOutput=False)
    bfc2_d = nc.declare_dram_parameter("bfc2", [2, P, 1], F32, isOutput=False)
    bkv_d = nc.declare_dram_parameter("bkv", [2, 2 * C], BF16, isOutput=False)
    bproj_d = nc.declare_dram_parameter("bproj", [1, C], BF16, isOutput=False)
    out_d = nc.declare_dram_parameter("out", [N, C], F32, isOutput=True)
    DBG = os.environ.get("BASS_DBG", "0") == "1"
    if DBG:
        dbg_d = {
            k: nc.declare_dram_parameter(f"dbg_{k}", shp, dt, isOutput=True)
            for k, (shp, dt) in {
                "xhT": ([P, 2 * N], BF16),
                "qT": ([P, 2 * N], FP8),
                "E": ([P, 2 * C], FP8),
                "h1": ([P, NTILES * C], F32),
                "x2T": ([P, 2 * N], FP8),
                "g1T": ([P, 8 * N], FP8),
                "mT": ([P, 2 * N], BF16),
            }.items()
        }

    with tile.TileContext(nc) as tc:
        import contextlib

        ctx = contextlib.ExitStack()
        with ctx:
            const = ctx.enter_context(tc.tile_pool(name="const", bufs=1))
            xres = ctx.enter_context(tc.tile_pool(name="xres", bufs=1))
            stats = ctx.enter_context(tc.tile_pool(name="stats", bufs=4))
            work = ctx.enter_context(tc.tile_pool(name="work", bufs=6))
            outp = ctx.enter_context(tc.tile_pool(name="outp", bufs=3))
            # transpose (bf16, 4 tiles batched) + c1 psum pool; lives all kernel
            ps_t = ctx.enter_context(tc.tile_pool(name="ps_t", bufs=1, space="PSUM"))

            # ---- residents ----
            x_sb = xres.tile([P, NTILES, C], F32)  # raw x, token-major
            h1_sb = xres.tile([P, NTILES, C], F32)  # x + attn, token-major
            xhT = xres.tile([P, 2, N], BF16)  # LN1(x)^T  (feature-major)
            qT8 = xres.tile([P, 2, N], FP8)  # (q*S_Q)^T
            x2T8 = xres.tile([P, 2, N], FP8)  # (LN2(h1)*S_X2)^T
            g1T8 = xres.tile([P, 8, N], FP8)  # gelu(fc1)^T
            mTb = xres.tile([P, 2, N], BF16)  # fc2 out, feature-major
            mv32 = xres.tile([P, NTILES, 2], F32)
            rs32 = xres.tile([P, NTILES], F32)
            nmr32 = xres.tile([P, NTILES], F32)  # -mean*rstd for LN2

            # ---- input DMAs: x first (compute starts on it), weights after --
            for g in range(NG):
                nc.sync.dma_start(
                    out=x_sb[:, 4 * g : 4 * g + 4, :],
                    in_=x_d[512 * g : 512 * (g + 1), :].rearrange(
                        "(s p) c -> p s c", p=P
                    ),
                )
            wkv = const.tile([P, 2, 2 * C], BF16)
            wq = const.tile([P, 2, C], BF16)
            wproj = const.tile([P, 2, C], BF16)
            bq = const.tile([P, 2], F32)
            nc.sync.dma_start(out=wkv[:], in_=wkv_d.rearrange("c p f -> p c f"))
            nc.sync.dma_start(out=wq[:], in_=wq_d.rearrange("c p f -> p c f"))
            nc.sync.dma_start(out=wproj[:], in_=wproj_d.rearrange("c p f -> p c f"))
            for c in range(2):
                nc.sync.dma_start(out=bq[:, c : c + 1], in_=bq_d[c])
            w18 = const.tile([P, 2, HID], FP8)
            w28 = const.tile([P, 8, C], FP8)
            b1 = const.tile([P, 8], F32)
            bfc2 = const.tile([P, 2], F32)
            nc.sync.dma_start(out=w18[:], in_=w1_d.rearrange("c p f -> p c f"))
            nc.sync.dma_start(out=w28[:], in_=w2_d.rearrange("c p f -> p c f"))
            for c in range(8):
                nc.sync.dma_start(out=b1[:, c : c + 1], in_=b1_d[c])
            for c in range(2):
                nc.sync.dma_start(out=bfc2[:, c : c + 1], in_=bfc2_d[c])
            ident = const.tile([P, P], BF16)
            masks.make_identity(nc, ident[:])
            ones_row = const.tile([1, P], BF16)
            nc.vector.memset(ones_row[:], 1.0)
            ones_col = const.tile([P, 1], BF16)
            nc.vector.memset(ones_col[:], 1.0)
            eps_t = const.tile([P, 1], F32)
            nc.vector.memset(eps_t[:], EPS)
            lnsx2_t = const.tile([P, 1], F32)
            nc.vector.memset(lnsx2_t[:], LN_SX2)
            bkv = const.tile([2, 2 * C], BF16)  # [bk|bv] ; row1 = bk_sv_nv row
            bproj = const.tile([1, C], BF16)
            if has_bkv:
                nc.sync.dma_start(out=bkv[:], in_=bkv_d[:])
            if has_bproj:
                nc.sync.dma_start(out=bproj[:], in_=bproj_d[:])

            # ---- helpers ----
            def ln_rstd(var_ap, mu_ap, rs_ap, nmr_ap, nsub, logmul_ap, tag):
                # rs = exp(-0.5*ln(var+eps) + logmul) == exp(logmul)/sqrt(var+eps)
                lnv = stats.tile([P, nsub], F32, tag=f"lnv{tag}")
                nc.scalar.activation(
                    out=lnv[:], in_=var_ap, func=AF.Ln, bias=eps_t[:]
                )
                if logmul_ap is None:
                    nc.scalar.activation(
                        out=rs_ap, in_=lnv[:], func=AF.Exp, scale=-0.5
                    )
                else:
                    nc.scalar.activation(
                        out=rs_ap, in_=lnv[:], func=AF.Exp, scale=-0.5,
                        bias=logmul_ap,
                    )
                # nmr = -mean * rs
                nc.vector.scalar_tensor_tensor(
                    out=nmr_ap, in0=mu_ap, scalar=-1.0, in1=rs_ap,
                    op0=ALU.mult, op1=ALU.mult,
                )

            def ln_normalize(src_ap, dst, rs1, nmr1, s):
                # dst = src*rs + (-mu*rs); alternate DVE / ACT
                if s % 2 == 0:
                    nc.vector.tensor_scalar(
                        out=dst, in0=src_ap, scalar1=rs1, scalar2=nmr1,
                        op0=ALU.mult, op1=ALU.add,
                    )
                else:
                    nc.scalar.activation(
                        out=dst, in_=src_ap, func=AF.Identity, scale=rs1,
                        bias=nmr1,
                    )

            # =============== Phase A: LN1, xhat^T, Gram accum, q^T ===========
            ab_ctx = contextlib.ExitStack()
            ps_G = ab_ctx.enter_context(
                tc.tile_pool(name="ps_G", bufs=1, space="PSUM")
            )
            G_ps = [ps_G.tile([P, C], F32, name=f"G{i}") for i in range(2)]
            s_ps = (
                [ps_G.tile([P, 1], F32, name=f"s{i}") for i in range(2)]
                if has_bkv
                else None
            )
            with tc.tile_pool(name="ps_q", bufs=2, space="PSUM") as ps_q:

                def q_pass(p):
                    # q^T for token chunks 4p..4p+3 (feature-major, fp8 out)
                    for fc in range(2):
                        for j in range(4):
                            ch = 4 * p + j
                            qps = ps_q.tile([P, 512], F32, tag="q")
                            for kc in range(2):
                                nc.tensor.matmul(
                                    qps[:],
                                    wq[:, kc, fc * P : (fc + 1) * P],
                                    xhT[:, kc, ch * 512 : (ch + 1) * 512],
                                    start=(kc == 0),
                                    stop=(kc == 1),
                                )
                            # qT8 = psum + bq_scaled   (cast fp8)
                            nc.scalar.activation(
                                out=qT8[:, fc, ch * 512 : (ch + 1) * 512],
                                in_=qps[:],
                                func=AF.Identity,
                                bias=bq[:, fc : fc + 1],
                            )

                for g in range(NG):
                    idxs = [4 * g + s for s in range(4)]
                    mv4 = stats.tile([P, 4, 2], F32, tag="mv")
                    rs4 = stats.tile([P, 4], F32, tag="rs")
                    nmr4 = stats.tile([P, 4], F32, tag="nmr")
                    for s, i in enumerate(idxs):
                        st = stats.tile([P, 6], F32, tag="bn")
                        nc.vector.bn_stats(out=st[:], in_=x_sb[:, i, :])
                        nc.vector.bn_aggr(out=mv4[:, s, :], in_=st[:])
                    ln_rstd(
                        mv4[:, :, 1], mv4[:, :, 0], rs4[:], nmr4[:], 4, None, "a"
                    )
                    tp4 = ps_t.tile([P, 4, C], BF16, tag="tp")
                    xh4 = [None] * 4
                    for s, i in enumerate(idxs):
                        xhat = work.tile([P, C], BF16, tag="xhat")
                        xh4[s] = xhat
                        ln_normalize(
                            x_sb[:, i, :], xhat[:], rs4[:, s : s + 1],
                            nmr4[:, s : s + 1], s,
                        )
                        for c in range(2):
                            nc.tensor.transpose(
                                tp4[:, s, c * P : (c + 1) * P],
                                xhat[:, c * P : (c + 1) * P],
                                ident[:],
                            )
                        # Gram accumulation: G[c] += xhat[:,c-half].T @ xhat
                        tile_i = 4 * g + s
                        for c in range(2):
                            nc.tensor.matmul(
                                G_ps[c][:],
                                xhat[:, c * P : (c + 1) * P],
                                xhat[:, :],
                                start=(tile_i == 0),
                                stop=(tile_i == NTILES - 1),
                            )
                        if has_bkv:
                            for c in range(2):
                                nc.tensor.matmul(
                                    s_ps[c][:],
                                    xhat[:, c * P : (c + 1) * P],
                                    ones_col[:],
                                    start=(tile_i == 0),
                                    stop=(tile_i == NTILES - 1),
                                )
                    # batched eviction of 4 transposed tiles -> xhT
                    nc.vector.tensor_copy(
                        out=xhT[:, :, g * 512 : (g + 1) * 512].rearrange(
                            "p c (s t) -> p c s t", s=4
                        ),
                        in_=tp4[:].rearrange("p s (c t) -> p c s t", c=2),
                    )
                    if g == 3:
                        q_pass(0)
                    elif g == 7:
                        q_pass(1)

            # =============== Phase B: logits = Wk^T G Wv, softmax -> E =======
            BdT = const.tile([P, 2, P], BF16)
            nc.vector.memset(BdT[:], 0.0)
            E8 = const.tile([P, 2, C], FP8)
            with tc.tile_pool(name="ps_b", bufs=2, space="PSUM") as ps_b, \
                 tc.tile_pool(name="ps_L", bufs=1, space="PSUM") as ps_L:
                g_sb = work.tile([P, 2, C], BF16, tag="gsb")
                for c in range(2):
                    nc.vector.tensor_copy(out=g_sb[:, c, :], in_=G_ps[c][:])
                # U = G @ Wv  (uses G symmetry: lhsT slice of g_sb)
                u_sb = work.tile([P, 2, C], BF16, tag="usb")
                for fo in range(2):
                    u_ps = ps_b.tile([P, C], F32, tag="b")
                    for cp in range(2):
                        nc.tensor.matmul(
                            u_ps[:],
                            g_sb[:, cp, fo * P : (fo + 1) * P],
                            wkv[:, cp, C : 2 * C],
                            start=(cp == 0),
                            stop=(cp == 1),
                        )
                    nc.vector.tensor_copy(out=u_sb[:, fo, :], in_=u_ps[:])
                # L = Wk^T U  -> one psum bank, halves at col 0/256
                L_tile = ps_L.tile([P, 2 * C], F32, name="L")
                L_ps = [L_tile[:, i * C : (i + 1) * C] for i in range(2)]
                for ko in range(2):
                    for cp in range(2):
                        nc.tensor.matmul(
                            L_ps[ko],
                            wkv[:, cp, ko * P : (ko + 1) * P],
                            u_sb[:, cp, :],
                            start=(cp == 0),
                            stop=(cp == 1),
                        )
                if has_bkv:
                    # bias corrections: L += bk (s^T Wv + N bv) + (Wk^T s) bv^T
                    s_sb = work.tile([P, 2], BF16, tag="ssb")
                    for c in range(2):
                        nc.vector.tensor_copy(out=s_sb[:, c : c + 1], in_=s_ps[c][:])
                    svr = work.tile([1, C], F32, tag="svr")
                    sv_ps = ps_b.tile([1, C], F32, tag="sv")
                    for cp in range(2):
                        nc.tensor.matmul(
                            sv_ps[:],
                            s_sb[:, cp : cp + 1],
                            wkv[:, cp, C : 2 * C],
                            start=(cp == 0),
                            stop=(cp == 1),
                        )
                    # svr = s^T Wv + N*bv   (bkv row1 holds N*bv host-side)
                    nc.vector.tensor_tensor(
                        out=svr[:], in0=sv_ps[:], in1=bkv[1:2, C : 2 * C],
                        op=ALU.add,
                    )
                    svr_b = work.tile([1, C], BF16, tag="svrb")
                    nc.vector.tensor_copy(out=svr_b[:], in_=svr[:])
                    sk_ps = ps_b.tile([1, C], F32, tag="sk")
                    for cp in range(2):
                        nc.tensor.matmul(
                            sk_ps[:],
                            s_sb[:, cp : cp + 1],
                            wkv[:, cp, 0:C],
                            start=(cp == 0),
                            stop=(cp == 1),
                        )
                    skr = work.tile([1, C], BF16, tag="skr")
                    nc.vector.tensor_copy(out=skr[:], in_=sk_ps[:])
                    for ko in range(2):
                        nc.tensor.matmul(
                            L_ps[ko],
                            bkv[0:1, C + ko * P : C + (ko + 1) * P],
                            svr_b[:],
                            start=False,
                            stop=False,
                        )
                        nc.tensor.matmul(
                            L_ps[ko],
                            skr[0:1, ko * P : (ko + 1) * P],
                            bkv[0:1, C : 2 * C],
                            start=False,
                            stop=True,
                        )
                for half in range(2):
                    a_sb = work.tile([P, HD], F32, tag="attn")
                    for h in range(4):
                        hh = half * 4 + h
                        nc.vector.tensor_copy(
                            out=a_sb[h * HD : (h + 1) * HD, :],
                            in_=L_tile[
                                h * HD : (h + 1) * HD,
                                half * C + hh * HD : half * C + (hh + 1) * HD,
                            ],
                        )
                    exps = work.tile([P, HD], F32, tag="exps")
                    nc.scalar.activation(
                        out=exps[:], in_=a_sb[:], func=AF.Exp
                    )
                    ssum = stats.tile([P, 1], F32, tag="ssum")
                    nc.vector.tensor_reduce(
                        out=ssum[:], in_=exps[:], axis=AX.X, op=ALU.add
                    )
                    rec = stats.tile([P, 1], F32, tag="rec")
                    nc.vector.reciprocal(out=rec[:], in_=ssum[:])
                    for h in range(4):
                        sl = slice(h * HD, (h + 1) * HD)
                        nc.vector.tensor_scalar(
                            out=BdT[sl, half, sl],
                            in0=exps[sl, :],
                            scalar1=rec[sl, 0:1],
                            scalar2=None,
                            op0=ALU.mult,
                        )
                for half in range(2):
                    e_ps = ps_b.tile([P, C], F32, tag="b")
                    nc.tensor.matmul(
                        e_ps[:],
                        BdT[:, half, :],
                        wproj[:, half, :],
                        start=True,
                        stop=True,
                    )
                    nc.vector.tensor_scalar(
                        out=E8[:, half, :],
                        in0=e_ps[:],
                        scalar1=S_E,
                        scalar2=None,
                        op0=ALU.mult,
                    )
            ab_ctx.close()  # free Gram psum banks before phase C pools open

            # =============== Phase C: proj+res+LN2 / fc1+gelu / fc2 ==========
            ps_f = ctx.enter_context(tc.tile_pool(name="ps_f", bufs=2, space="PSUM"))
            ps_m = ctx.enter_context(tc.tile_pool(name="ps_m", bufs=1, space="PSUM"))

            def c1_pair(i):
                # proj + residual for tiles i, i+1 in one [P,512] psum tile
                p_ps = ps_t.tile([P, 512], F32, tag="c1", name=f"pp{i}")
                for u in range(2):
                    nc.tensor.matmul(
                        p_ps[:, u * C : (u + 1) * C],
                        qT8[:, :, (i + u) * P : (i + u + 1) * P],
                        E8[:, :, :],
                        start=True,
                        stop=not has_bproj,
                        perf_mode=DR,
                    )
                    if has_bproj:
                        nc.tensor.matmul(
                            p_ps[:, u * C : (u + 1) * C],
                            ones_row[:],
                            bproj[:],
                            start=False,
                            stop=True,
                        )
                # h1 = x + proj_out  (f32, token-major), 2 tiles at once
                nc.vector.scalar_tensor_tensor(
                    out=h1_sb[:, i : i + 2, :],
                    in0=p_ps[:].rearrange("p (u c) -> p u c", u=2),
                    scalar=INV_PROJ,
                    in1=x_sb[:, i : i + 2, :],
                    op0=ALU.mult,
                    op1=ALU.add,
                )
                for u in range(2):
                    st = stats.tile([P, 6], F32, tag="bn", name=f"st{i + u}")
                    nc.vector.bn_stats(out=st[:], in_=h1_sb[:, i + u, :])
                    nc.vector.bn_aggr(out=mv32[:, i + u, :], in_=st[:])

            def ln2_group(g):
                tp4 = ps_t.tile([P, 4, C], BF16, tag="tp")
                for s in range(4):
                    i = 4 * g + s
                    x2 = work.tile([P, C], BF16, tag="x2")
                    ln_normalize(
                        h1_sb[:, i, :], x2[:], rs32[:, i : i + 1],
                        nmr32[:, i : i + 1], s,
                    )
                    for c in range(2):
                        nc.tensor.transpose(
                            tp4[:, s, c * P : (c + 1) * P],
                            x2[:, c * P : (c + 1) * P],
                            ident[:],
                        )
                # batched eviction -> x2T8 (fp8)
                nc.scalar.copy(
                    out=x2T8[:, :, g * 512 : (g + 1) * 512].rearrange(
                        "p c (s t) -> p c s t", s=4
                    ),
                    in_=tp4[:].rearrange("p s (c t) -> p c s t", c=2),
                )

            def fc1_half(hf):
                # hidden rows, fp8 DoubleRow; 2 token-quarters per half
                for hc in range(8):
                    for tq in range(2):
                        q0 = (2 * hf + tq) * 1024
                        f_ps = ps_f.tile([P, 1024], F32, tag="f")
                        for u in range(2):
                            nc.tensor.matmul(
                                f_ps[:, u * 512 : (u + 1) * 512],
                                w18[:, :, hc * P : (hc + 1) * P],
                                x2T8[:, :, q0 + u * 512 : q0 + (u + 1) * 512],
                                start=True,
                                stop=True,
                                perf_mode=DR,
                            )
                        nc.scalar.activation(
                            out=g1T8[:, hc, q0 : q0 + 1024],
                            in_=f_ps[:],
                            func=AF.Gelu,
                            bias=b1[:, hc : hc + 1],
                            scale=1.0 / (S_X2 * s_w1),
                        )

            def fc2_half(hf):
                # feature-major: stationary = w2 pair slices
                for cs in range(2):
                    for ph in range(2):
                        mps = [
                            ps_m.tile(
                                [P, 512], F32, tag=f"m{j}",
                                name=f"mp{hf}{cs}{ph}{j}",
                            )
                            for j in range(2)
                        ]
                        for j in range(4):
                            for u in range(2):
                                tch = 2 * ph + u
                                t0 = (4 * hf + tch) * 512
                                nc.tensor.matmul(
                                    mps[u][:],
                                    w28[:, 2 * j : 2 * j + 2, cs * P : (cs + 1) * P],
                                    g1T8[:, 2 * j : 2 * j + 2, t0 : t0 + 512],
                                    start=(j == 0),
                                    stop=(j == 3),
                                    perf_mode=DR,
                                )
                        for u in range(2):
                            tch = 2 * ph + u
                            t0 = (4 * hf + tch) * 512
                            nc.vector.tensor_scalar(
                                out=mTb[:, cs, t0 : t0 + 512],
                                in0=mps[u][:],
                                scalar1=1.0 / s_w2,
                                scalar2=bfc2[:, cs : cs + 1],
                                op0=ALU.mult,
                                op1=ALU.add,
                            )

            def out_half(hf):
                for g in range(4 * hf, 4 * hf + 4):
                    och = outp.tile([P, 4, C], F32, tag="oc")
                    tp4 = ps_t.tile([P, 4, C], BF16, tag="tp")
                    for s in range(4):
                        i = 4 * g + s
                        for c in range(2):
                            nc.tensor.transpose(
                                tp4[:, s, c * P : (c + 1) * P],
                                mTb[:, c, i * P : (i + 1) * P],
                                ident[:],
                            )
                    t1 = outp.tile([P, 4, C], F32, tag="t1")
                    nc.vector.tensor_tensor(
                        out=t1[:],
                        in0=tp4[:],
                        in1=h1_sb[:, 4 * g : 4 * g + 4, :],
                        op=ALU.add,
                    )
                    nc.gpsimd.tensor_tensor(
                        out=och[:], in0=t1[:], in1=x_sb[:, 4 * g : 4 * g + 4, :],
                        op=ALU.add,
                    )
                    nc.sync.dma_start(
                        out=out_d[512 * g : 512 * (g + 1), :].rearrange(
                            "(s p) c -> p s c", p=P
                        ),
                        in_=och[:],
                    )

            def half_rstd(hf):
                sl = slice(16 * hf, 16 * (hf + 1))
                ln_rstd(
                    mv32[:, sl, 1], mv32[:, sl, 0], rs32[:, sl], nmr32[:, sl],
                    16, lnsx2_t[:], f"c{hf}",
                )

            # C1 for all tiles first (keeps ln/exp ACT table resident), then
            # the MLP pipeline in two half-N passes: fc1(H0)'s gelu window
            # overlaps LN2(H1); fc1(H1)'s overlaps fc2(H0)+outputs(H0).
            for i in range(0, 16, 2):
                c1_pair(i)
            half_rstd(0)
            for i in range(16, 32, 2):
                c1_pair(i)
            half_rstd(1)
            for g in range(4):
                ln2_group(g)
            fc1_half(0)
            for g in range(4, 8):
                ln2_group(g)
            fc2_half(0)
            out_half(0)
            fc1_half(1)
            fc2_half(1)
            out_half(1)

            if DBG:
                for k, src in {
                    "xhT": xhT,
                    "qT": qT8,
                    "E": E8,
                    "h1": h1_sb,
                    "x2T": x2T8,
                    "g1T": g1T8,
                    "mT": mTb,
                }.items():
                    nc.sync.dma_start(
                        out=dbg_d[k][:], in_=src[:].rearrange("p a b -> p (a b)")
                    )

    _split_sync_waits(nc)
    return nc


_CACHE = {}


def _get_nc(key):
    if key not in _CACHE:
        _CACHE[key] = _build_nc(*key)
    return _CACHE[key]


def _pow2_floor(x):
    return float(2.0 ** np.floor(np.log2(x)))


def _prep_inputs(inputs):
    f32 = lambda k: np.asarray(inputs[k], dtype=np.float32)
    qkv_w, qkv_b = f32("qkv_w"), f32("qkv_b")
    proj_w, proj_b = f32("proj_w"), f32("proj_b")
    ln1_g, ln1_b = f32("ln1_g"), f32("ln1_b")
    ln2_g, ln2_b = f32("ln2_g"), f32("ln2_b")
    fc1_w, fc1_b = f32("fc1_w"), f32("fc1_b")
    fc2_w, fc2_b = f32("fc2_w"), f32("fc2_b")

    scale = HD ** (-0.5)

    # Fold LN1 affine into qkv: LN1(x)@W+b = xhat@(g*W) + (ln1_b@W + b)
    wqkv_f = ln1_g[:, None] * qkv_w
    bqkv_f = ln1_b @ qkv_w + qkv_b
    # Fold channel-attention scale into k; q gets its fp8 range scale
    wk = wqkv_f[:, C : 2 * C] * scale
    wv = wqkv_f[:, 2 * C : 3 * C]
    wqs = wqkv_f[:, 0:C] * S_Q
    bk = bqkv_f[C : 2 * C] * scale
    bv = bqkv_f[2 * C : 3 * C]
    bqs = bqkv_f[0:C] * S_Q
    # Fold LN2 affine into fc1
    w1_f = ln2_g[:, None] * fc1_w
    b1_f = ln2_b @ fc1_w + fc1_b

    s_w1 = _pow2_floor(FP8_SAFE / max(np.abs(w1_f).max(), 1e-30))
    s_w2 = _pow2_floor(FP8_SAFE / max(np.abs(fc2_w).max(), 1e-30))

    wkv = np.concatenate([wk, wv], axis=1)  # [256, 512]
    bkv_row0 = np.concatenate([bk, bv])
    bkv_row1 = np.concatenate([np.zeros(C, np.float32), N * bv])
    bkv = np.stack([bkv_row0, bkv_row1])

    has_bkv = bool(np.any(bkv_row0 != 0))
    has_bproj = bool(np.any(proj_b != 0))

    shared = {
        "wkv": wkv.reshape(2, P, 2 * C).astype(NP_BF16),
        "wq": wqs.reshape(2, P, C).astype(NP_BF16),
        "wproj": proj_w.reshape(2, P, C).astype(NP_BF16),
        "w1": (w1_f * s_w1).reshape(2, P, HID).astype(NP_FP8),
        "w2": (fc2_w * s_w2).reshape(8, P, C).astype(NP_FP8),
        "bq": bqs.reshape(2, P, 1).astype(np.float32),
        "b1": b1_f.reshape(8, P, 1).astype(np.float32),
        "bfc2": fc2_b.reshape(2, P, 1).astype(np.float32),
        "bkv": bkv.astype(NP_BF16),
        "bproj": proj_b.reshape(1, C).astype(NP_BF16),
    }
    return shared, (has_bkv, has_bproj, s_w1, s_w2)


def kernel(x, **weights):
    x = np.asarray(x, dtype=np.float32)
    shared, key = _prep_inputs(weights)
    nc = _get_nc(key)
    in_maps = [dict(shared, x=np.ascontiguousarray(x[b])) for b in range(B)]
    res = run_bass_kernel_spmd(nc, in_maps, list(range(B)))
    out = np.stack([res.results[b]["out"] for b in range(B)], axis=0)
    return out.astype(np.float32)
